# revision 28
# baseline (speedup 1.0000x reference)
"""DeepPoly SPU transformer — Trainium2 Bass kernel (custom-DVE edition).

Elementwise over N=16777216; sharded across 8 NeuronCores (2M elems each,
viewed as [nt x 128 x fd] fp16).  All wire traffic is fp16 (24MB/core round
trip = ~69.9us at the 360GB/s DMA roofline); the three input streams are
packed into one DRAM tensor (and the three outputs into another) so each
chunk needs one input DMA (+ a split [nl|nu]/[out] pair of output DMAs so
the early streams can fill DMA gaps).  The whole per-element DAG is
collapsed into 3 fused custom-DVE ops + a DVE tensor-tensor max + 2 ACT
transcendentals + an ACT/Pool tail for the `out` stream, so every engine
sits below the DMA roofline (~8.7us per 128x2048 chunk): per full chunk
ACT 7.6us, DVE 7.7us, Pool 6.9us vs DMA 8.7us.  The first tile is split
into quarter/half chunks (and routed through the low-latency all-DVE out
path) to shorten the pipeline fill; measured 75187ns vs the ~73.3us
model floor (first-DMA latency + DMA busy + final sem prop).

Math (per element; Z = sqrt(0.5), spu(t) = t^2-0.5 for t>=0 else
sigmoid(-t)-1).  Device emits nl/nu in doubled space (host applies the
constant affine epilogue  nl = 0.5*nlD - 0.5,  nu = 0.5*nuD):

  sld = tanh(-l/2) = 2*(sigmoid(-l) - 0.5)      [ACT; sign(sld) = -sign(l)]
  sgx = sigmoid(x)                              [ACT]
  out = relu(x)^2 - min(sgx, 0.5)               [ACT relu/square + Pool]
  P2  = 2*g2m*(l - g2m/4),  g2m = 2Z*max([l<0], (u+l)/(2Z))   [custom DVE:
        G1 = max([l<0], (u+l)/(2Z));  P2 = G1*(4Z*l - G1)]
  SD  = sld*[u<=0] - BIG*([u>=Z] + [sld<=0])    [custom DVE]
  nuD = max(2*relu(u)^2, sld) + (2*[sld<=0]-1)  [custom DVE]
  nlD = max(P2, SD)                             [DVE tt max]

Case boundaries (l vs 0, u vs 0, u vs Z) are pinned to the f32 side during
host-side fp16 conversion; l<0 is additionally kept <= -2.5e-7 so that
tanh(-l/2) cannot round to +-0 in fp16 (sld's sign carries [l<0] on
device).  Validated vs the f64 reference: relmax_vs_scale ~ 1e-3 on all
three outputs (tolerance 2e-2).
"""

import numpy as np

import concourse.bass as bass
import concourse.bacc as bacc
import concourse.mybir as mybir
from concourse.tile import TileContext
from concourse.bass_utils import run_bass_kernel_spmd
import concourse.dve_ops as dve_ops
from concourse.dve_spec import (
    Spec, Src0, Src1, C0, C1, Zero, One, maxx, minn, relu, sq, lower as _dve_lower,
    _has_src1,
)
from concourse.dve_uop import DveOpSpec

_N = 16777216
_NCORES = 8
_P = 128
_FDT = _N // _NCORES // _P  # 16384 free elems per partition per core

_Z32 = np.float32(np.sqrt(0.5))
_Z16 = float(np.float16(np.sqrt(0.5)))
_INV2Z = float(np.float32(1.0 / (2.0 * float(_Z32))))
_FOURZ = float(np.float32(4.0 * float(_Z32)))
_BIG = 1000.0

_AF = mybir.ActivationFunctionType
_OP = mybir.AluOpType
_F16 = mybir.dt.float16


# --------------------------------------------------------------------------
# custom DVE ops (registered once per process via the documented extension
# point in concourse.dve_ops; names are namespaced to this kernel)
# --------------------------------------------------------------------------

def _register(name, spec):
    if name in dve_ops._SUB_OPCODE_FOR_NAME:
        for op in dve_ops.OPS:
            if op.name == name:
                return op
    row = max(dve_ops._SUB_OPCODE_FOR_NAME.values()) + 1
    assert row < 0x20, "custom-DVE opcode rows exhausted"
    shas = {}
    for ver in ("v3", "v4"):
        u = _dve_lower(spec, ver=ver)
        shas[ver] = DveOpSpec(
            name=name, opcode=row, uops=u, rd1_en=_has_src1(spec)
        ).sha(ver)
    op = dve_ops.DveOp(name, spec, subdim=False, uops_sha=shas)
    dve_ops.OPS.append(op)
    dve_ops._SUB_OPCODE_FOR_NAME[name] = row
    dve_ops.CUSTOM_DVE_SPECS[name] = spec
    return op


def _ref_p2(in0, in1, c0, c1, c2):
    l = in0.astype(np.float32)
    u = in1.astype(np.float32)
    s2s = (l + u) * np.float32(c0)
    g1 = np.maximum((l < 0).astype(np.float32), s2s)
    return g1 * (l * np.float32(c1) - g1)


_g1 = maxx(Src0 < Zero, (Src0 + Src1) * C0)
_SPEC_P2 = Spec(body=_g1 * (Src0 * C1 - _g1), reference=_ref_p2)


def _ref_sd(in0, in1, c0, c1, c2):
    u = in0.astype(np.float32)
    sld = in1.astype(np.float32)
    m1 = (u <= 0).astype(np.float32)
    m2 = (u >= np.float32(c0)).astype(np.float32)
    m3 = (sld <= 0).astype(np.float32)
    return sld * m1 + (m2 + m3) * np.float32(c1)


_SPEC_SD = Spec(
    body=Src1 * (Src0 <= Zero) + ((Src0 >= C0) + (Src1 <= Zero)) * C1,
    reference=_ref_sd,
)


def _ref_nu(in0, in1, c0, c1, c2):
    u = in0.astype(np.float32)
    sld = in1.astype(np.float32)
    r2 = np.maximum(u, 0) ** 2
    c = (sld <= 0).astype(np.float32)
    return np.maximum(r2 + r2, sld) + (c + c - 1.0)


_r2 = sq(relu(Src0))
_c = (Src1 <= Zero)
_SPEC_NU = Spec(body=maxx(_r2 + _r2, Src1) + (_c + _c - One), reference=_ref_nu)


def _ref_out(in0, in1, c0, c1, c2):
    x = in0.astype(np.float32)
    sgx = in1.astype(np.float32)
    return np.maximum(x, 0) ** 2 - np.minimum(sgx, np.float32(c0))


_SPEC_OUT = Spec(body=sq(relu(Src0)) - minn(Src1, C0), reference=_ref_out)

_OP_P2 = _register("SPU_P2_ANT", _SPEC_P2)
_OP_SD = _register("SPU_SD_ANT", _SPEC_SD)
_OP_NU = _register("SPU_NU_ANT", _SPEC_NU)
_OP_OUT = _register("SPU_OUT_ANT", _SPEC_OUT)


# --------------------------------------------------------------------------
# kernel build
# --------------------------------------------------------------------------

def _build_nc(fd=2048, io_bufs=4, out_bufs=4, tmp_bufs=3, ramp="s24e2",
              dve_head=2, dve_tail=0, rx_dve=0, in_q="sp", out_q="sp",
              emit_order="dma_first", skew=3, hp_out=0, split_out=1, split_in=0,
              half_out=0, hp_act=0, hp_max=0, sg_dve=0, act_order=0, dve_every=0,
              rx_set=None, fdt=_FDT):
    """dve_head/dve_tail: how many chunks at each end of the pipeline route
    their `out` stream through the fused DVE op (low latency) instead of the
    ACT/Pool tail (better steady-state balance).  rx_dve: chunks whose
    relu(x) runs as a DVE tensor_scalar instead of ACT."""
    nc = bacc.Bacc(trn_type="TRN2", debug=False, num_devices=_NCORES)
    nt = fdt // fd

    def _q(name):
        return {"sp": nc.sync, "act": nc.scalar, "dve": nc.vector,
                "pool": nc.gpsimd}[name]

    t_in = nc.dram_tensor("pin", [nt, _P, 3, fd], _F16, kind="ExternalInput")
    t_out = nc.dram_tensor("pout", [nt, _P, 3, fd], _F16, kind="ExternalOutput")

    with TileContext(nc) as tc:
        with tc.tile_pool(name="io", bufs=io_bufs) as iop, \
             tc.tile_pool(name="ot", bufs=out_bufs) as otp, \
             tc.tile_pool(name="tmp", bufs=tmp_bufs) as tp:

            if ramp == "end":
                chunks = [(i, 0, fd) for i in range(nt - 1)]
                chunks += [(nt - 1, c, fd // 2) for c in range(0, fd, fd // 2)]
            elif ramp == "end4":
                chunks = [(i, 0, fd) for i in range(nt - 1)]
                chunks += [(nt - 1, c, fd // 4) for c in range(0, fd, fd // 4)]
            elif ramp == "both":
                chunks = [(0, c, fd // 2) for c in range(0, fd, fd // 2)]
                chunks += [(i, 0, fd) for i in range(1, nt - 1)]
                chunks += [(nt - 1, c, fd // 4) for c in range(0, fd, fd // 4)]
            elif ramp == "s4e2":
                chunks = [(0, c, fd // 4) for c in range(0, fd, fd // 4)]
                chunks += [(i, 0, fd) for i in range(1, nt - 1)]
                chunks += [(nt - 1, c, fd // 2) for c in range(0, fd, fd // 2)]
            elif ramp == "s4e4":
                chunks = [(0, c, fd // 4) for c in range(0, fd, fd // 4)]
                chunks += [(i, 0, fd) for i in range(1, nt - 1)]
                chunks += [(nt - 1, c, fd // 4) for c in range(0, fd, fd // 4)]
            elif ramp == "s2e2":
                chunks = [(0, c, fd // 2) for c in range(0, fd, fd // 2)]
                chunks += [(i, 0, fd) for i in range(1, nt - 1)]
                chunks += [(nt - 1, c, fd // 2) for c in range(0, fd, fd // 2)]
            elif ramp == "s24e2":
                chunks = [(0, 0, fd // 4), (0, fd // 4, fd // 4), (0, fd // 2, fd // 2)]
                chunks += [(i, 0, fd) for i in range(1, nt - 1)]
                chunks += [(nt - 1, c, fd // 2) for c in range(0, fd, fd // 2)]
            elif ramp == "s24e24":
                chunks = [(0, 0, fd // 4), (0, fd // 4, fd // 4), (0, fd // 2, fd // 2)]
                chunks += [(i, 0, fd) for i in range(1, nt - 1)]
                chunks += [(nt - 1, 0, fd // 2), (nt - 1, fd // 2, fd // 4),
                           (nt - 1, 3 * fd // 4, fd // 4)]
            elif ramp.startswith("m"):
                # s24 start, split tile <m> (and optionally more) into halves
                msp = {int(c) for c in ramp[1:].split("_")}
                chunks = [(0, 0, fd // 4), (0, fd // 4, fd // 4), (0, fd // 2, fd // 2)]
                for i in range(1, nt - 1):
                    if i in msp:
                        chunks += [(i, 0, fd // 2), (i, fd // 2, fd // 2)]
                    else:
                        chunks.append((i, 0, fd))
                chunks += [(nt - 1, c, fd // 2) for c in range(0, fd, fd // 2)]
            elif ramp == "s42e2":
                chunks = [(0, 0, fd // 2), (0, fd // 2, fd // 4),
                          (0, 3 * fd // 4, fd // 4)]
                chunks += [(i, 0, fd) for i in range(1, nt - 1)]
                chunks += [(nt - 1, c, fd // 2) for c in range(0, fd, fd // 2)]
            elif ramp == "s124e2":
                chunks = [(0, 0, fd // 8), (0, fd // 8, fd // 8),
                          (0, fd // 4, fd // 4), (0, fd // 2, fd // 2)]
                chunks += [(i, 0, fd) for i in range(1, nt - 1)]
                chunks += [(nt - 1, c, fd // 2) for c in range(0, fd, fd // 2)]
            elif ramp == "s24":
                chunks = [(0, 0, fd // 4), (0, fd // 4, fd // 4), (0, fd // 2, fd // 2)]
                chunks += [(i, 0, fd) for i in range(1, nt)]
            elif ramp == "s24e2x":
                chunks = [(0, 0, fd // 4), (0, fd // 4, fd // 4), (0, fd // 2, fd // 2)]
                chunks += [(i, 0, fd) for i in range(1, nt - 2)]
                chunks += [(i, c, fd // 2) for i in (nt - 2, nt - 1)
                           for c in range(0, fd, fd // 2)]
            elif ramp == "s8e2":
                chunks = [(0, c, fd // 8) for c in range(0, fd, fd // 8)]
                chunks += [(i, 0, fd) for i in range(1, nt - 1)]
                chunks += [(nt - 1, c, fd // 2) for c in range(0, fd, fd // 2)]
            else:
                chunks = [(i, 0, fd) for i in range(nt)]

            n = len(chunks)
            st = [None] * n

            def SDMA(ci):
                i, c0, fdc = chunks[ci]
                with tc.high_priority():
                    it = iop.tile([_P, 3 * fdc], _F16, tag="in")
                    itv = it[:].rearrange("p (s f) -> p s f", s=3)
                    if split_in:
                        _q(in_q).dma_start(out=itv[:, 0:2, :],
                                           in_=t_in[i, :, 0:2, c0:c0 + fdc])
                        _q(in_q).dma_start(out=itv[:, 2:3, :],
                                           in_=t_in[i, :, 2:3, c0:c0 + fdc])
                    else:
                        _q(in_q).dma_start(out=itv, in_=t_in[i, :, :, c0:c0 + fdc])
                st[ci] = dict(it=it)

            def S0(ci):
                i, c0, fdc = chunks[ci]
                it = st[ci]["it"]
                l = it[:, 0:fdc]
                x = it[:, 2 * fdc:3 * fdc]
                sld = tp.tile([_P, fdc], _F16, tag="sld")
                sgx = tp.tile([_P, fdc], _F16, tag="sgx")
                if hp_act:
                    with tc.high_priority():
                        nc.scalar.activation(sld[:], l, _AF.Tanh, scale=-0.5)
                        nc.scalar.activation(sgx[:], x, _AF.Sigmoid)
                elif act_order == 1:
                    nc.scalar.activation(sgx[:], x, _AF.Sigmoid)
                    nc.scalar.activation(sld[:], l, _AF.Tanh, scale=-0.5)
                elif act_order == 2:
                    pass  # emitted after rx2 below
                else:
                    nc.scalar.activation(sld[:], l, _AF.Tanh, scale=-0.5)
                    nc.scalar.activation(sgx[:], x, _AF.Sigmoid)
                dve_out = (ci < dve_head or ci >= n - dve_tail or
                           (dve_every and ci % dve_every == 0))
                if act_order == 2 and dve_out:
                    nc.scalar.activation(sld[:], l, _AF.Tanh, scale=-0.5)
                    nc.scalar.activation(sgx[:], x, _AF.Sigmoid)
                if not dve_out:
                    h = fdc // 2 if (half_out and fdc == fd) else 0
                    rx2 = tp.tile([_P, fdc - h], _F16, tag="rx2")
                    if (rx_set is not None and ci in rx_set) or \
                            (rx_dve and (ci % rx_dve == 0)):
                        rx = tp.tile([_P, fdc - h], _F16, tag="rx")
                        nc.vector.tensor_scalar(rx[:], x[:, h:fdc], 0.0, None, _OP.max)
                        nc.scalar.activation(rx2[:], rx[:], _AF.Square)
                    else:
                        nc.scalar.activation(rx2[:], x[:, h:fdc], _AF.Relu)
                        if act_order == 2:
                            nc.scalar.activation(sld[:], l, _AF.Tanh, scale=-0.5)
                            nc.scalar.activation(sgx[:], x, _AF.Sigmoid)
                        nc.scalar.activation(rx2[:], rx2[:], _AF.Square)
                    st[ci].update(rx2=rx2, h=h)
                st[ci].update(sld=sld, sgx=sgx, dve_out=dve_out)

            def S1(ci):
                i, c0, fdc = chunks[ci]
                d = st[ci]
                it = d["it"]
                l3 = it[:, 0:fdc].unsqueeze(1)
                u3 = it[:, fdc:2 * fdc].unsqueeze(1)
                sld3 = d["sld"][:].unsqueeze(1)
                ot = otp.tile([_P, 3 * fdc], _F16, tag="out")
                p2 = tp.tile([_P, fdc], _F16, tag="p2")
                sd = tp.tile([_P, fdc], _F16, tag="sd")
                nc.vector._custom_dve(_OP_P2, out=p2[:].unsqueeze(1), in0=l3,
                                      in1=u3, s0=_INV2Z, s1=_FOURZ)
                nc.vector._custom_dve(_OP_SD, out=sd[:].unsqueeze(1), in0=u3,
                                      in1=sld3, s0=_Z16, s1=-_BIG)
                nc.vector._custom_dve(_OP_NU, out=ot[:, 2 * fdc:3 * fdc].unsqueeze(1),
                                      in0=u3, in1=sld3)
                if d["dve_out"]:
                    x3 = it[:, 2 * fdc:3 * fdc].unsqueeze(1)
                    nc.vector._custom_dve(_OP_OUT, out=ot[:, 0:fdc].unsqueeze(1),
                                          in0=x3, in1=d["sgx"][:].unsqueeze(1),
                                          s0=0.5)
                else:
                    h = d["h"]
                    if h:
                        x3h = it[:, 2 * fdc:2 * fdc + h].unsqueeze(1)
                        nc.vector._custom_dve(
                            _OP_OUT, out=ot[:, 0:h].unsqueeze(1), in0=x3h,
                            in1=d["sgx"][:, 0:h].unsqueeze(1), s0=0.5)
                    sg = tp.tile([_P, fdc - h], _F16, tag="sg")
                    eng = nc.vector if sg_dve else nc.gpsimd
                    eng.tensor_scalar(sg[:], d["sgx"][:, h:fdc], 0.5, None,
                                      _OP.min)
                    st[ci].update(sg=sg)
                st[ci].update(ot=ot, p2=p2, sd=sd)

            def S2(ci):
                i, c0, fdc = chunks[ci]
                d = st[ci]
                ot = d["ot"]
                if not d["dve_out"]:
                    h = d["h"]
                    nc.gpsimd.tensor_tensor(ot[:, h:fdc], d["rx2"][:], d["sg"][:],
                                            _OP.subtract)
                if hp_max:
                    with tc.high_priority():
                        nc.vector.tensor_tensor(ot[:, fdc:2 * fdc], d["p2"][:],
                                                d["sd"][:], _OP.max)
                else:
                    nc.vector.tensor_tensor(ot[:, fdc:2 * fdc], d["p2"][:],
                                            d["sd"][:], _OP.max)
                if split_out and not d["dve_out"]:
                    otv = ot[:].rearrange("p (s f) -> p s f", s=3)
                    _q(out_q).dma_start(
                        out=t_out[i, :, 1:3, c0:c0 + fdc],
                        in_=otv[:, 1:3, :])
                    _q(out_q).dma_start(
                        out=t_out[i, :, 0:1, c0:c0 + fdc],
                        in_=otv[:, 0:1, :])
                elif hp_out:
                    with tc.high_priority():
                        _q(out_q).dma_start(
                            out=t_out[i, :, :, c0:c0 + fdc],
                            in_=ot[:].rearrange("p (s f) -> p s f", s=3))
                else:
                    _q(out_q).dma_start(
                        out=t_out[i, :, :, c0:c0 + fdc],
                        in_=ot[:].rearrange("p (s f) -> p s f", s=3))
                st[ci] = None

            if skew == 0:
                for ci in range(n):
                    SDMA(ci); S0(ci); S1(ci); S2(ci)
            elif skew == 1:
                for k in range(n + 1):
                    if k < n: SDMA(k)
                    if 0 <= k - 1 < n:
                        S0(k - 1); S1(k - 1); S2(k - 1)
            elif skew == 2:
                for k in range(n + 2):
                    if k < n: SDMA(k)
                    if 0 <= k - 1 < n: S0(k - 1)
                    if 0 <= k - 2 < n: S1(k - 2); S2(k - 2)
            else:
                order = {
                    "dma_first": lambda k: [(SDMA, k), (S0, k - 1), (S1, k - 2), (S2, k - 3)],
                    "s2_mid": lambda k: [(SDMA, k), (S0, k - 1), (S2, k - 3), (S1, k - 2)],
                }[emit_order]
                for k in range(n + 3):
                    for fn, ci in order(k):
                        if 0 <= ci < n:
                            fn(ci)

    nc.compile()
    return nc


_NC_CACHE = {}


def _get_nc(**kw):
    key = tuple(sorted(kw.items()))
    if key not in _NC_CACHE:
        _NC_CACHE[key] = _build_nc(**kw)
    return _NC_CACHE[key]


def _prep_inputs(x, lower_bounds, upper_bounds):
    """fp16 conversion with case-boundary pinning (see module docstring)."""
    F16 = np.float16
    x16 = x.astype(F16)
    l16 = lower_bounds.astype(F16)
    u16 = upper_bounds.astype(F16)
    # l<0 must stay strictly negative AND large enough that tanh(-l/2)
    # cannot round to zero in fp16 (sld's sign carries [l<0] on device).
    l16 = np.where((lower_bounds < 0) & (l16 >= -2.5e-7), F16(-2.5e-7), l16)
    # u>0 must stay strictly positive (case selection uses u<=0).
    u16 = np.where((upper_bounds > 0) & (u16 <= 0), F16(6e-8), u16)
    # u vs Z: the reference jumps at u==Z; keep each element on its f32 side.
    z16 = F16(_Z16)
    below = np.nextafter(z16, F16(0))
    u16 = np.where((upper_bounds >= _Z32) & (u16 < z16), z16, u16)
    u16 = np.where((upper_bounds < _Z32) & (u16 >= z16), below, u16)
    return x16, l16, u16


def _run(x, lower_bounds, upper_bounds, trace=False, **build_kw):
    assert x.shape == (_N,) and x.dtype == np.float32
    nc = _get_nc(**build_kw)
    fd = build_kw.get("fd", 2048)
    nt = _FDT // fd
    x16, l16, u16 = _prep_inputs(x, lower_bounds, upper_bounds)
    shp = (_NCORES, nt, _P, fd)
    packed = np.empty((_NCORES, nt, _P, 3, fd), dtype=np.float16)
    packed[..., 0, :] = l16.reshape(shp)
    packed[..., 1, :] = u16.reshape(shp)
    packed[..., 2, :] = x16.reshape(shp)
    in_maps = [{"pin": packed[c]} for c in range(_NCORES)]
    res = run_bass_kernel_spmd(
        nc, in_maps, core_ids=list(range(_NCORES)), trace=trace
    )
    pout = np.stack([res.results[c]["pout"] for c in range(_NCORES)])
    out = np.ascontiguousarray(pout[..., 0, :]).reshape(-1).astype(np.float32)
    nl = np.ascontiguousarray(pout[..., 1, :]).reshape(-1).astype(np.float32)
    nl = nl * 0.5 - 0.5  # device emits nl in doubled +0.5 space
    nu = np.ascontiguousarray(pout[..., 2, :]).reshape(-1).astype(np.float32)
    nu = nu * 0.5  # device emits nu in doubled space
    return (out, nl, nu), res


def kernel(x, lower_bounds, upper_bounds):
    (out, nl, nu), _ = _run(x, lower_bounds, upper_bounds)
    return (out, nl, nu)


# revision 29
# speedup vs baseline: 1.0039x; 1.0039x over previous
"""DeepPoly SPU transformer — Trainium2 Bass kernel (custom-DVE edition).

Elementwise over N=16777216; sharded across 8 NeuronCores (2M elems each,
viewed as [nt x 128 x fd] fp16).  All wire traffic is fp16 (24MB/core round
trip = ~69.9us at the 360GB/s DMA roofline); the three input streams are
packed into one DRAM tensor (and the three outputs into another) so each
chunk needs one input DMA (+ a split [nl|nu]/[out] pair of output DMAs so
the early streams can fill DMA gaps).  The whole per-element DAG is
collapsed into 3 fused custom-DVE ops + a DVE tensor-tensor max + 2 ACT
transcendentals + an ACT/Pool tail for the `out` stream, so every engine
sits below the DMA roofline (~8.7us per 128x2048 chunk): per full chunk
ACT 7.6us, DVE 7.7us, Pool 6.9us vs DMA 8.7us.  The first tile is split
into quarter/half chunks (and routed through the low-latency all-DVE out
path) to shorten the pipeline fill; the nl max runs at high priority so
its out-DMA is never stuck behind the next chunk's fused ops.  Measured
74894ns vs the ~73.3us model floor (first-DMA latency + DMA busy + final
sem prop).

Math (per element; Z = sqrt(0.5), spu(t) = t^2-0.5 for t>=0 else
sigmoid(-t)-1).  Device emits nl/nu in doubled space (host applies the
constant affine epilogue  nl = 0.5*nlD - 0.5,  nu = 0.5*nuD):

  sld = tanh(-l/2) = 2*(sigmoid(-l) - 0.5)      [ACT; sign(sld) = -sign(l)]
  sgx = sigmoid(x)                              [ACT]
  out = relu(x)^2 - min(sgx, 0.5)               [ACT relu/square + Pool]
  P2  = 2*g2m*(l - g2m/4),  g2m = 2Z*max([l<0], (u+l)/(2Z))   [custom DVE:
        G1 = max([l<0], (u+l)/(2Z));  P2 = G1*(4Z*l - G1)]
  SD  = sld*[u<=0] - BIG*([u>=Z] + [sld<=0])    [custom DVE]
  nuD = max(2*relu(u)^2, sld) + (2*[sld<=0]-1)  [custom DVE]
  nlD = max(P2, SD)                             [DVE tt max]

Case boundaries (l vs 0, u vs 0, u vs Z) are pinned to the f32 side during
host-side fp16 conversion; l<0 is additionally kept <= -2.5e-7 so that
tanh(-l/2) cannot round to +-0 in fp16 (sld's sign carries [l<0] on
device).  Validated vs the f64 reference: relmax_vs_scale ~ 1e-3 on all
three outputs (tolerance 2e-2).
"""

import numpy as np

import concourse.bass as bass
import concourse.bacc as bacc
import concourse.mybir as mybir
from concourse.tile import TileContext
from concourse.bass_utils import run_bass_kernel_spmd
import concourse.dve_ops as dve_ops
from concourse.dve_spec import (
    Spec, Src0, Src1, C0, C1, Zero, One, maxx, minn, relu, sq, lower as _dve_lower,
    _has_src1,
)
from concourse.dve_uop import DveOpSpec

_N = 16777216
_NCORES = 8
_P = 128
_FDT = _N // _NCORES // _P  # 16384 free elems per partition per core

_Z32 = np.float32(np.sqrt(0.5))
_Z16 = float(np.float16(np.sqrt(0.5)))
_INV2Z = float(np.float32(1.0 / (2.0 * float(_Z32))))
_FOURZ = float(np.float32(4.0 * float(_Z32)))
_BIG = 1000.0

_AF = mybir.ActivationFunctionType
_OP = mybir.AluOpType
_F16 = mybir.dt.float16


# --------------------------------------------------------------------------
# custom DVE ops (registered once per process via the documented extension
# point in concourse.dve_ops; names are namespaced to this kernel)
# --------------------------------------------------------------------------

def _register(name, spec):
    if name in dve_ops._SUB_OPCODE_FOR_NAME:
        for op in dve_ops.OPS:
            if op.name == name:
                return op
    row = max(dve_ops._SUB_OPCODE_FOR_NAME.values()) + 1
    assert row < 0x20, "custom-DVE opcode rows exhausted"
    shas = {}
    for ver in ("v3", "v4"):
        u = _dve_lower(spec, ver=ver)
        shas[ver] = DveOpSpec(
            name=name, opcode=row, uops=u, rd1_en=_has_src1(spec)
        ).sha(ver)
    op = dve_ops.DveOp(name, spec, subdim=False, uops_sha=shas)
    dve_ops.OPS.append(op)
    dve_ops._SUB_OPCODE_FOR_NAME[name] = row
    dve_ops.CUSTOM_DVE_SPECS[name] = spec
    return op


def _ref_p2(in0, in1, c0, c1, c2):
    l = in0.astype(np.float32)
    u = in1.astype(np.float32)
    s2s = (l + u) * np.float32(c0)
    g1 = np.maximum((l < 0).astype(np.float32), s2s)
    return g1 * (l * np.float32(c1) - g1)


_g1 = maxx(Src0 < Zero, (Src0 + Src1) * C0)
_SPEC_P2 = Spec(body=_g1 * (Src0 * C1 - _g1), reference=_ref_p2)


def _ref_sd(in0, in1, c0, c1, c2):
    u = in0.astype(np.float32)
    sld = in1.astype(np.float32)
    m1 = (u <= 0).astype(np.float32)
    m2 = (u >= np.float32(c0)).astype(np.float32)
    m3 = (sld <= 0).astype(np.float32)
    return sld * m1 + (m2 + m3) * np.float32(c1)


_SPEC_SD = Spec(
    body=Src1 * (Src0 <= Zero) + ((Src0 >= C0) + (Src1 <= Zero)) * C1,
    reference=_ref_sd,
)


def _ref_nu(in0, in1, c0, c1, c2):
    u = in0.astype(np.float32)
    sld = in1.astype(np.float32)
    r2 = np.maximum(u, 0) ** 2
    c = (sld <= 0).astype(np.float32)
    return np.maximum(r2 + r2, sld) + (c + c - 1.0)


_r2 = sq(relu(Src0))
_c = (Src1 <= Zero)
_SPEC_NU = Spec(body=maxx(_r2 + _r2, Src1) + (_c + _c - One), reference=_ref_nu)


def _ref_out(in0, in1, c0, c1, c2):
    x = in0.astype(np.float32)
    sgx = in1.astype(np.float32)
    return np.maximum(x, 0) ** 2 - np.minimum(sgx, np.float32(c0))


_SPEC_OUT = Spec(body=sq(relu(Src0)) - minn(Src1, C0), reference=_ref_out)

_OP_P2 = _register("SPU_P2_ANT", _SPEC_P2)
_OP_SD = _register("SPU_SD_ANT", _SPEC_SD)
_OP_NU = _register("SPU_NU_ANT", _SPEC_NU)
_OP_OUT = _register("SPU_OUT_ANT", _SPEC_OUT)


# --------------------------------------------------------------------------
# kernel build
# --------------------------------------------------------------------------

def _build_nc(fd=2048, io_bufs=4, out_bufs=4, tmp_bufs=2, ramp="s24e2",
              dve_head=2, dve_tail=0, rx_dve=0, in_q="sp", out_q="sp",
              emit_order="dma_first", skew=3, hp_out=0, split_out=1, split_in=0,
              half_out=0, hp_act=0, hp_max=1, sg_dve=0, act_order=0, dve_every=0,
              rx_set=None, fdt=_FDT):
    """dve_head/dve_tail: how many chunks at each end of the pipeline route
    their `out` stream through the fused DVE op (low latency) instead of the
    ACT/Pool tail (better steady-state balance).  rx_dve: chunks whose
    relu(x) runs as a DVE tensor_scalar instead of ACT."""
    nc = bacc.Bacc(trn_type="TRN2", debug=False, num_devices=_NCORES)
    nt = fdt // fd

    def _q(name):
        return {"sp": nc.sync, "act": nc.scalar, "dve": nc.vector,
                "pool": nc.gpsimd}[name]

    t_in = nc.dram_tensor("pin", [nt, _P, 3, fd], _F16, kind="ExternalInput")
    t_out = nc.dram_tensor("pout", [nt, _P, 3, fd], _F16, kind="ExternalOutput")

    with TileContext(nc) as tc:
        with tc.tile_pool(name="io", bufs=io_bufs) as iop, \
             tc.tile_pool(name="ot", bufs=out_bufs) as otp, \
             tc.tile_pool(name="tmp", bufs=tmp_bufs) as tp:

            if ramp == "end":
                chunks = [(i, 0, fd) for i in range(nt - 1)]
                chunks += [(nt - 1, c, fd // 2) for c in range(0, fd, fd // 2)]
            elif ramp == "end4":
                chunks = [(i, 0, fd) for i in range(nt - 1)]
                chunks += [(nt - 1, c, fd // 4) for c in range(0, fd, fd // 4)]
            elif ramp == "both":
                chunks = [(0, c, fd // 2) for c in range(0, fd, fd // 2)]
                chunks += [(i, 0, fd) for i in range(1, nt - 1)]
                chunks += [(nt - 1, c, fd // 4) for c in range(0, fd, fd // 4)]
            elif ramp == "s4e2":
                chunks = [(0, c, fd // 4) for c in range(0, fd, fd // 4)]
                chunks += [(i, 0, fd) for i in range(1, nt - 1)]
                chunks += [(nt - 1, c, fd // 2) for c in range(0, fd, fd // 2)]
            elif ramp == "s4e4":
                chunks = [(0, c, fd // 4) for c in range(0, fd, fd // 4)]
                chunks += [(i, 0, fd) for i in range(1, nt - 1)]
                chunks += [(nt - 1, c, fd // 4) for c in range(0, fd, fd // 4)]
            elif ramp == "s2e2":
                chunks = [(0, c, fd // 2) for c in range(0, fd, fd // 2)]
                chunks += [(i, 0, fd) for i in range(1, nt - 1)]
                chunks += [(nt - 1, c, fd // 2) for c in range(0, fd, fd // 2)]
            elif ramp == "s24e2":
                chunks = [(0, 0, fd // 4), (0, fd // 4, fd // 4), (0, fd // 2, fd // 2)]
                chunks += [(i, 0, fd) for i in range(1, nt - 1)]
                chunks += [(nt - 1, c, fd // 2) for c in range(0, fd, fd // 2)]
            elif ramp == "s24e24":
                chunks = [(0, 0, fd // 4), (0, fd // 4, fd // 4), (0, fd // 2, fd // 2)]
                chunks += [(i, 0, fd) for i in range(1, nt - 1)]
                chunks += [(nt - 1, 0, fd // 2), (nt - 1, fd // 2, fd // 4),
                           (nt - 1, 3 * fd // 4, fd // 4)]
            elif ramp.startswith("m"):
                # s24 start, split tile <m> (and optionally more) into halves
                msp = {int(c) for c in ramp[1:].split("_")}
                chunks = [(0, 0, fd // 4), (0, fd // 4, fd // 4), (0, fd // 2, fd // 2)]
                for i in range(1, nt - 1):
                    if i in msp:
                        chunks += [(i, 0, fd // 2), (i, fd // 2, fd // 2)]
                    else:
                        chunks.append((i, 0, fd))
                chunks += [(nt - 1, c, fd // 2) for c in range(0, fd, fd // 2)]
            elif ramp == "s42e2":
                chunks = [(0, 0, fd // 2), (0, fd // 2, fd // 4),
                          (0, 3 * fd // 4, fd // 4)]
                chunks += [(i, 0, fd) for i in range(1, nt - 1)]
                chunks += [(nt - 1, c, fd // 2) for c in range(0, fd, fd // 2)]
            elif ramp == "s124e2":
                chunks = [(0, 0, fd // 8), (0, fd // 8, fd // 8),
                          (0, fd // 4, fd // 4), (0, fd // 2, fd // 2)]
                chunks += [(i, 0, fd) for i in range(1, nt - 1)]
                chunks += [(nt - 1, c, fd // 2) for c in range(0, fd, fd // 2)]
            elif ramp == "s24":
                chunks = [(0, 0, fd // 4), (0, fd // 4, fd // 4), (0, fd // 2, fd // 2)]
                chunks += [(i, 0, fd) for i in range(1, nt)]
            elif ramp == "s24e2x":
                chunks = [(0, 0, fd // 4), (0, fd // 4, fd // 4), (0, fd // 2, fd // 2)]
                chunks += [(i, 0, fd) for i in range(1, nt - 2)]
                chunks += [(i, c, fd // 2) for i in (nt - 2, nt - 1)
                           for c in range(0, fd, fd // 2)]
            elif ramp == "s8e2":
                chunks = [(0, c, fd // 8) for c in range(0, fd, fd // 8)]
                chunks += [(i, 0, fd) for i in range(1, nt - 1)]
                chunks += [(nt - 1, c, fd // 2) for c in range(0, fd, fd // 2)]
            else:
                chunks = [(i, 0, fd) for i in range(nt)]

            n = len(chunks)
            st = [None] * n

            def SDMA(ci):
                i, c0, fdc = chunks[ci]
                with tc.high_priority():
                    it = iop.tile([_P, 3 * fdc], _F16, tag="in")
                    itv = it[:].rearrange("p (s f) -> p s f", s=3)
                    if split_in:
                        _q(in_q).dma_start(out=itv[:, 0:2, :],
                                           in_=t_in[i, :, 0:2, c0:c0 + fdc])
                        _q(in_q).dma_start(out=itv[:, 2:3, :],
                                           in_=t_in[i, :, 2:3, c0:c0 + fdc])
                    else:
                        _q(in_q).dma_start(out=itv, in_=t_in[i, :, :, c0:c0 + fdc])
                st[ci] = dict(it=it)

            def S0(ci):
                i, c0, fdc = chunks[ci]
                it = st[ci]["it"]
                l = it[:, 0:fdc]
                x = it[:, 2 * fdc:3 * fdc]
                sld = tp.tile([_P, fdc], _F16, tag="sld")
                sgx = tp.tile([_P, fdc], _F16, tag="sgx")
                if hp_act:
                    with tc.high_priority():
                        nc.scalar.activation(sld[:], l, _AF.Tanh, scale=-0.5)
                        nc.scalar.activation(sgx[:], x, _AF.Sigmoid)
                elif act_order == 1:
                    nc.scalar.activation(sgx[:], x, _AF.Sigmoid)
                    nc.scalar.activation(sld[:], l, _AF.Tanh, scale=-0.5)
                elif act_order == 2:
                    pass  # emitted after rx2 below
                else:
                    nc.scalar.activation(sld[:], l, _AF.Tanh, scale=-0.5)
                    nc.scalar.activation(sgx[:], x, _AF.Sigmoid)
                dve_out = (ci < dve_head or ci >= n - dve_tail or
                           (dve_every and ci % dve_every == 0))
                if act_order == 2 and dve_out:
                    nc.scalar.activation(sld[:], l, _AF.Tanh, scale=-0.5)
                    nc.scalar.activation(sgx[:], x, _AF.Sigmoid)
                if not dve_out:
                    h = fdc // 2 if (half_out and fdc == fd) else 0
                    rx2 = tp.tile([_P, fdc - h], _F16, tag="rx2")
                    if (rx_set is not None and ci in rx_set) or \
                            (rx_dve and (ci % rx_dve == 0)):
                        rx = tp.tile([_P, fdc - h], _F16, tag="rx")
                        nc.vector.tensor_scalar(rx[:], x[:, h:fdc], 0.0, None, _OP.max)
                        nc.scalar.activation(rx2[:], rx[:], _AF.Square)
                    else:
                        nc.scalar.activation(rx2[:], x[:, h:fdc], _AF.Relu)
                        if act_order == 2:
                            nc.scalar.activation(sld[:], l, _AF.Tanh, scale=-0.5)
                            nc.scalar.activation(sgx[:], x, _AF.Sigmoid)
                        nc.scalar.activation(rx2[:], rx2[:], _AF.Square)
                    st[ci].update(rx2=rx2, h=h)
                st[ci].update(sld=sld, sgx=sgx, dve_out=dve_out)

            def S1(ci):
                i, c0, fdc = chunks[ci]
                d = st[ci]
                it = d["it"]
                l3 = it[:, 0:fdc].unsqueeze(1)
                u3 = it[:, fdc:2 * fdc].unsqueeze(1)
                sld3 = d["sld"][:].unsqueeze(1)
                ot = otp.tile([_P, 3 * fdc], _F16, tag="out")
                p2 = tp.tile([_P, fdc], _F16, tag="p2")
                sd = tp.tile([_P, fdc], _F16, tag="sd")
                nc.vector._custom_dve(_OP_P2, out=p2[:].unsqueeze(1), in0=l3,
                                      in1=u3, s0=_INV2Z, s1=_FOURZ)
                nc.vector._custom_dve(_OP_SD, out=sd[:].unsqueeze(1), in0=u3,
                                      in1=sld3, s0=_Z16, s1=-_BIG)
                nc.vector._custom_dve(_OP_NU, out=ot[:, 2 * fdc:3 * fdc].unsqueeze(1),
                                      in0=u3, in1=sld3)
                if d["dve_out"]:
                    x3 = it[:, 2 * fdc:3 * fdc].unsqueeze(1)
                    nc.vector._custom_dve(_OP_OUT, out=ot[:, 0:fdc].unsqueeze(1),
                                          in0=x3, in1=d["sgx"][:].unsqueeze(1),
                                          s0=0.5)
                else:
                    h = d["h"]
                    if h:
                        x3h = it[:, 2 * fdc:2 * fdc + h].unsqueeze(1)
                        nc.vector._custom_dve(
                            _OP_OUT, out=ot[:, 0:h].unsqueeze(1), in0=x3h,
                            in1=d["sgx"][:, 0:h].unsqueeze(1), s0=0.5)
                    sg = tp.tile([_P, fdc - h], _F16, tag="sg")
                    eng = nc.vector if sg_dve else nc.gpsimd
                    eng.tensor_scalar(sg[:], d["sgx"][:, h:fdc], 0.5, None,
                                      _OP.min)
                    st[ci].update(sg=sg)
                st[ci].update(ot=ot, p2=p2, sd=sd)

            def S2(ci):
                i, c0, fdc = chunks[ci]
                d = st[ci]
                ot = d["ot"]
                if not d["dve_out"]:
                    h = d["h"]
                    nc.gpsimd.tensor_tensor(ot[:, h:fdc], d["rx2"][:], d["sg"][:],
                                            _OP.subtract)
                if hp_max:
                    with tc.high_priority():
                        nc.vector.tensor_tensor(ot[:, fdc:2 * fdc], d["p2"][:],
                                                d["sd"][:], _OP.max)
                else:
                    nc.vector.tensor_tensor(ot[:, fdc:2 * fdc], d["p2"][:],
                                            d["sd"][:], _OP.max)
                if split_out and not d["dve_out"]:
                    otv = ot[:].rearrange("p (s f) -> p s f", s=3)
                    _q(out_q).dma_start(
                        out=t_out[i, :, 1:3, c0:c0 + fdc],
                        in_=otv[:, 1:3, :])
                    _q(out_q).dma_start(
                        out=t_out[i, :, 0:1, c0:c0 + fdc],
                        in_=otv[:, 0:1, :])
                elif hp_out:
                    with tc.high_priority():
                        _q(out_q).dma_start(
                            out=t_out[i, :, :, c0:c0 + fdc],
                            in_=ot[:].rearrange("p (s f) -> p s f", s=3))
                else:
                    _q(out_q).dma_start(
                        out=t_out[i, :, :, c0:c0 + fdc],
                        in_=ot[:].rearrange("p (s f) -> p s f", s=3))
                st[ci] = None

            if skew == 0:
                for ci in range(n):
                    SDMA(ci); S0(ci); S1(ci); S2(ci)
            elif skew == 1:
                for k in range(n + 1):
                    if k < n: SDMA(k)
                    if 0 <= k - 1 < n:
                        S0(k - 1); S1(k - 1); S2(k - 1)
            elif skew == 2:
                for k in range(n + 2):
                    if k < n: SDMA(k)
                    if 0 <= k - 1 < n: S0(k - 1)
                    if 0 <= k - 2 < n: S1(k - 2); S2(k - 2)
            else:
                order = {
                    "dma_first": lambda k: [(SDMA, k), (S0, k - 1), (S1, k - 2), (S2, k - 3)],
                    "s2_mid": lambda k: [(SDMA, k), (S0, k - 1), (S2, k - 3), (S1, k - 2)],
                }[emit_order]
                for k in range(n + 3):
                    for fn, ci in order(k):
                        if 0 <= ci < n:
                            fn(ci)

    nc.compile()
    return nc


_NC_CACHE = {}


def _get_nc(**kw):
    key = tuple(sorted(kw.items()))
    if key not in _NC_CACHE:
        _NC_CACHE[key] = _build_nc(**kw)
    return _NC_CACHE[key]


def _prep_inputs(x, lower_bounds, upper_bounds):
    """fp16 conversion with case-boundary pinning (see module docstring)."""
    F16 = np.float16
    x16 = x.astype(F16)
    l16 = lower_bounds.astype(F16)
    u16 = upper_bounds.astype(F16)
    # l<0 must stay strictly negative AND large enough that tanh(-l/2)
    # cannot round to zero in fp16 (sld's sign carries [l<0] on device).
    l16 = np.where((lower_bounds < 0) & (l16 >= -2.5e-7), F16(-2.5e-7), l16)
    # u>0 must stay strictly positive (case selection uses u<=0).
    u16 = np.where((upper_bounds > 0) & (u16 <= 0), F16(6e-8), u16)
    # u vs Z: the reference jumps at u==Z; keep each element on its f32 side.
    z16 = F16(_Z16)
    below = np.nextafter(z16, F16(0))
    u16 = np.where((upper_bounds >= _Z32) & (u16 < z16), z16, u16)
    u16 = np.where((upper_bounds < _Z32) & (u16 >= z16), below, u16)
    return x16, l16, u16


def _run(x, lower_bounds, upper_bounds, trace=False, **build_kw):
    assert x.shape == (_N,) and x.dtype == np.float32
    nc = _get_nc(**build_kw)
    fd = build_kw.get("fd", 2048)
    nt = _FDT // fd
    x16, l16, u16 = _prep_inputs(x, lower_bounds, upper_bounds)
    shp = (_NCORES, nt, _P, fd)
    packed = np.empty((_NCORES, nt, _P, 3, fd), dtype=np.float16)
    packed[..., 0, :] = l16.reshape(shp)
    packed[..., 1, :] = u16.reshape(shp)
    packed[..., 2, :] = x16.reshape(shp)
    in_maps = [{"pin": packed[c]} for c in range(_NCORES)]
    res = run_bass_kernel_spmd(
        nc, in_maps, core_ids=list(range(_NCORES)), trace=trace
    )
    pout = np.stack([res.results[c]["pout"] for c in range(_NCORES)])
    out = np.ascontiguousarray(pout[..., 0, :]).reshape(-1).astype(np.float32)
    nl = np.ascontiguousarray(pout[..., 1, :]).reshape(-1).astype(np.float32)
    nl = nl * 0.5 - 0.5  # device emits nl in doubled +0.5 space
    nu = np.ascontiguousarray(pout[..., 2, :]).reshape(-1).astype(np.float32)
    nu = nu * 0.5  # device emits nu in doubled space
    return (out, nl, nu), res


def kernel(x, lower_bounds, upper_bounds):
    (out, nl, nu), _ = _run(x, lower_bounds, upper_bounds)
    return (out, nl, nu)


# revision 30
# speedup vs baseline: 1.0172x; 1.0133x over previous
"""DeepPoly SPU transformer — Trainium2 Bass kernel (custom-DVE edition).

Elementwise over N=16777216; sharded across 8 NeuronCores (2M elems each,
viewed as [nt x 128 x fd] fp16).  All wire traffic is fp16 (24MB/core round
trip = ~69.9us at the 360GB/s DMA roofline); the three input streams are
packed into one DRAM tensor (and the three outputs into another) so each
chunk needs one input DMA (+ a split [nl|nu]/[out] pair of output DMAs so
the early streams can fill DMA gaps).  The whole per-element DAG is
collapsed into 3 fused custom-DVE ops + a DVE tensor-tensor max + 2 ACT
transcendentals + an ACT/Pool tail for the `out` stream, so every engine
sits below the DMA roofline (~8.7us per 128x2048 chunk): per full chunk
ACT 7.6us, DVE 7.7us, Pool 6.9us vs DMA 8.7us.  The first tile is split
into quarter/half chunks (and routed through the low-latency all-DVE out
path) to shorten the pipeline fill; the nl max runs at high priority so
its out-DMA is never stuck behind the next chunk's fused ops.  Measured
74894ns vs the ~73.3us model floor (first-DMA latency + DMA busy + final
sem prop); 73913ns with the joint-sweep config (io_bufs=5, tmp_bufs=2,
high-priority nl-max and out-DMAs — the knobs only win in combination).

Math (per element; Z = sqrt(0.5), spu(t) = t^2-0.5 for t>=0 else
sigmoid(-t)-1).  Device emits nl/nu in doubled space (host applies the
constant affine epilogue  nl = 0.5*nlD - 0.5,  nu = 0.5*nuD):

  sld = tanh(-l/2) = 2*(sigmoid(-l) - 0.5)      [ACT; sign(sld) = -sign(l)]
  sgx = sigmoid(x)                              [ACT]
  out = relu(x)^2 - min(sgx, 0.5)               [ACT relu/square + Pool]
  P2  = 2*g2m*(l - g2m/4),  g2m = 2Z*max([l<0], (u+l)/(2Z))   [custom DVE:
        G1 = max([l<0], (u+l)/(2Z));  P2 = G1*(4Z*l - G1)]
  SD  = sld*[u<=0] - BIG*([u>=Z] + [sld<=0])    [custom DVE]
  nuD = max(2*relu(u)^2, sld) + (2*[sld<=0]-1)  [custom DVE]
  nlD = max(P2, SD)                             [DVE tt max]

Case boundaries (l vs 0, u vs 0, u vs Z) are pinned to the f32 side during
host-side fp16 conversion; l<0 is additionally kept <= -2.5e-7 so that
tanh(-l/2) cannot round to +-0 in fp16 (sld's sign carries [l<0] on
device).  Validated vs the f64 reference: relmax_vs_scale ~ 1e-3 on all
three outputs (tolerance 2e-2).
"""

import numpy as np

import concourse.bass as bass
import concourse.bacc as bacc
import concourse.mybir as mybir
from concourse.tile import TileContext
from concourse.bass_utils import run_bass_kernel_spmd
import concourse.dve_ops as dve_ops
from concourse.dve_spec import (
    Spec, Src0, Src1, C0, C1, Zero, One, maxx, minn, relu, sq, lower as _dve_lower,
    _has_src1,
)
from concourse.dve_uop import DveOpSpec

_N = 16777216
_NCORES = 8
_P = 128
_FDT = _N // _NCORES // _P  # 16384 free elems per partition per core

_Z32 = np.float32(np.sqrt(0.5))
_Z16 = float(np.float16(np.sqrt(0.5)))
_INV2Z = float(np.float32(1.0 / (2.0 * float(_Z32))))
_FOURZ = float(np.float32(4.0 * float(_Z32)))
_BIG = 1000.0

_AF = mybir.ActivationFunctionType
_OP = mybir.AluOpType
_F16 = mybir.dt.float16


# --------------------------------------------------------------------------
# custom DVE ops (registered once per process via the documented extension
# point in concourse.dve_ops; names are namespaced to this kernel)
# --------------------------------------------------------------------------

def _register(name, spec):
    if name in dve_ops._SUB_OPCODE_FOR_NAME:
        for op in dve_ops.OPS:
            if op.name == name:
                return op
    row = max(dve_ops._SUB_OPCODE_FOR_NAME.values()) + 1
    assert row < 0x20, "custom-DVE opcode rows exhausted"
    shas = {}
    for ver in ("v3", "v4"):
        u = _dve_lower(spec, ver=ver)
        shas[ver] = DveOpSpec(
            name=name, opcode=row, uops=u, rd1_en=_has_src1(spec)
        ).sha(ver)
    op = dve_ops.DveOp(name, spec, subdim=False, uops_sha=shas)
    dve_ops.OPS.append(op)
    dve_ops._SUB_OPCODE_FOR_NAME[name] = row
    dve_ops.CUSTOM_DVE_SPECS[name] = spec
    return op


def _ref_p2(in0, in1, c0, c1, c2):
    l = in0.astype(np.float32)
    u = in1.astype(np.float32)
    s2s = (l + u) * np.float32(c0)
    g1 = np.maximum((l < 0).astype(np.float32), s2s)
    return g1 * (l * np.float32(c1) - g1)


_g1 = maxx(Src0 < Zero, (Src0 + Src1) * C0)
_SPEC_P2 = Spec(body=_g1 * (Src0 * C1 - _g1), reference=_ref_p2)


def _ref_sd(in0, in1, c0, c1, c2):
    u = in0.astype(np.float32)
    sld = in1.astype(np.float32)
    m1 = (u <= 0).astype(np.float32)
    m2 = (u >= np.float32(c0)).astype(np.float32)
    m3 = (sld <= 0).astype(np.float32)
    return sld * m1 + (m2 + m3) * np.float32(c1)


_SPEC_SD = Spec(
    body=Src1 * (Src0 <= Zero) + ((Src0 >= C0) + (Src1 <= Zero)) * C1,
    reference=_ref_sd,
)


def _ref_nu(in0, in1, c0, c1, c2):
    u = in0.astype(np.float32)
    sld = in1.astype(np.float32)
    r2 = np.maximum(u, 0) ** 2
    c = (sld <= 0).astype(np.float32)
    return np.maximum(r2 + r2, sld) + (c + c - 1.0)


_r2 = sq(relu(Src0))
_c = (Src1 <= Zero)
_SPEC_NU = Spec(body=maxx(_r2 + _r2, Src1) + (_c + _c - One), reference=_ref_nu)


def _ref_out(in0, in1, c0, c1, c2):
    x = in0.astype(np.float32)
    sgx = in1.astype(np.float32)
    return np.maximum(x, 0) ** 2 - np.minimum(sgx, np.float32(c0))


_SPEC_OUT = Spec(body=sq(relu(Src0)) - minn(Src1, C0), reference=_ref_out)

_OP_P2 = _register("SPU_P2_ANT", _SPEC_P2)
_OP_SD = _register("SPU_SD_ANT", _SPEC_SD)
_OP_NU = _register("SPU_NU_ANT", _SPEC_NU)
_OP_OUT = _register("SPU_OUT_ANT", _SPEC_OUT)


# --------------------------------------------------------------------------
# kernel build
# --------------------------------------------------------------------------

def _build_nc(fd=2048, io_bufs=5, out_bufs=4, tmp_bufs=2, ramp="s24e2",
              dve_head=2, dve_tail=0, rx_dve=0, in_q="sp", out_q="sp",
              emit_order="dma_first", skew=3, hp_out=1, split_out=1, split_in=0,
              half_out=0, hp_act=0, hp_max=1, sg_dve=0, act_order=0, dve_every=0,
              rx_set=None, fdt=_FDT):
    """dve_head/dve_tail: how many chunks at each end of the pipeline route
    their `out` stream through the fused DVE op (low latency) instead of the
    ACT/Pool tail (better steady-state balance).  rx_dve: chunks whose
    relu(x) runs as a DVE tensor_scalar instead of ACT."""
    nc = bacc.Bacc(trn_type="TRN2", debug=False, num_devices=_NCORES)
    nt = fdt // fd

    def _q(name):
        return {"sp": nc.sync, "act": nc.scalar, "dve": nc.vector,
                "pool": nc.gpsimd}[name]

    t_in = nc.dram_tensor("pin", [nt, _P, 3, fd], _F16, kind="ExternalInput")
    t_out = nc.dram_tensor("pout", [nt, _P, 3, fd], _F16, kind="ExternalOutput")

    with TileContext(nc) as tc:
        with tc.tile_pool(name="io", bufs=io_bufs) as iop, \
             tc.tile_pool(name="ot", bufs=out_bufs) as otp, \
             tc.tile_pool(name="tmp", bufs=tmp_bufs) as tp:

            if ramp == "end":
                chunks = [(i, 0, fd) for i in range(nt - 1)]
                chunks += [(nt - 1, c, fd // 2) for c in range(0, fd, fd // 2)]
            elif ramp == "end4":
                chunks = [(i, 0, fd) for i in range(nt - 1)]
                chunks += [(nt - 1, c, fd // 4) for c in range(0, fd, fd // 4)]
            elif ramp == "both":
                chunks = [(0, c, fd // 2) for c in range(0, fd, fd // 2)]
                chunks += [(i, 0, fd) for i in range(1, nt - 1)]
                chunks += [(nt - 1, c, fd // 4) for c in range(0, fd, fd // 4)]
            elif ramp == "s4e2":
                chunks = [(0, c, fd // 4) for c in range(0, fd, fd // 4)]
                chunks += [(i, 0, fd) for i in range(1, nt - 1)]
                chunks += [(nt - 1, c, fd // 2) for c in range(0, fd, fd // 2)]
            elif ramp == "s4e4":
                chunks = [(0, c, fd // 4) for c in range(0, fd, fd // 4)]
                chunks += [(i, 0, fd) for i in range(1, nt - 1)]
                chunks += [(nt - 1, c, fd // 4) for c in range(0, fd, fd // 4)]
            elif ramp == "s2e2":
                chunks = [(0, c, fd // 2) for c in range(0, fd, fd // 2)]
                chunks += [(i, 0, fd) for i in range(1, nt - 1)]
                chunks += [(nt - 1, c, fd // 2) for c in range(0, fd, fd // 2)]
            elif ramp == "s24e2":
                chunks = [(0, 0, fd // 4), (0, fd // 4, fd // 4), (0, fd // 2, fd // 2)]
                chunks += [(i, 0, fd) for i in range(1, nt - 1)]
                chunks += [(nt - 1, c, fd // 2) for c in range(0, fd, fd // 2)]
            elif ramp == "s24e24":
                chunks = [(0, 0, fd // 4), (0, fd // 4, fd // 4), (0, fd // 2, fd // 2)]
                chunks += [(i, 0, fd) for i in range(1, nt - 1)]
                chunks += [(nt - 1, 0, fd // 2), (nt - 1, fd // 2, fd // 4),
                           (nt - 1, 3 * fd // 4, fd // 4)]
            elif ramp.startswith("m"):
                # s24 start, split tile <m> (and optionally more) into halves
                msp = {int(c) for c in ramp[1:].split("_")}
                chunks = [(0, 0, fd // 4), (0, fd // 4, fd // 4), (0, fd // 2, fd // 2)]
                for i in range(1, nt - 1):
                    if i in msp:
                        chunks += [(i, 0, fd // 2), (i, fd // 2, fd // 2)]
                    else:
                        chunks.append((i, 0, fd))
                chunks += [(nt - 1, c, fd // 2) for c in range(0, fd, fd // 2)]
            elif ramp == "s42e2":
                chunks = [(0, 0, fd // 2), (0, fd // 2, fd // 4),
                          (0, 3 * fd // 4, fd // 4)]
                chunks += [(i, 0, fd) for i in range(1, nt - 1)]
                chunks += [(nt - 1, c, fd // 2) for c in range(0, fd, fd // 2)]
            elif ramp == "s124e2":
                chunks = [(0, 0, fd // 8), (0, fd // 8, fd // 8),
                          (0, fd // 4, fd // 4), (0, fd // 2, fd // 2)]
                chunks += [(i, 0, fd) for i in range(1, nt - 1)]
                chunks += [(nt - 1, c, fd // 2) for c in range(0, fd, fd // 2)]
            elif ramp == "s24":
                chunks = [(0, 0, fd // 4), (0, fd // 4, fd // 4), (0, fd // 2, fd // 2)]
                chunks += [(i, 0, fd) for i in range(1, nt)]
            elif ramp == "s24e2x":
                chunks = [(0, 0, fd // 4), (0, fd // 4, fd // 4), (0, fd // 2, fd // 2)]
                chunks += [(i, 0, fd) for i in range(1, nt - 2)]
                chunks += [(i, c, fd // 2) for i in (nt - 2, nt - 1)
                           for c in range(0, fd, fd // 2)]
            elif ramp == "s8e2":
                chunks = [(0, c, fd // 8) for c in range(0, fd, fd // 8)]
                chunks += [(i, 0, fd) for i in range(1, nt - 1)]
                chunks += [(nt - 1, c, fd // 2) for c in range(0, fd, fd // 2)]
            else:
                chunks = [(i, 0, fd) for i in range(nt)]

            n = len(chunks)
            st = [None] * n

            def SDMA(ci):
                i, c0, fdc = chunks[ci]
                with tc.high_priority():
                    it = iop.tile([_P, 3 * fdc], _F16, tag="in")
                    itv = it[:].rearrange("p (s f) -> p s f", s=3)
                    if split_in:
                        _q(in_q).dma_start(out=itv[:, 0:2, :],
                                           in_=t_in[i, :, 0:2, c0:c0 + fdc])
                        _q(in_q).dma_start(out=itv[:, 2:3, :],
                                           in_=t_in[i, :, 2:3, c0:c0 + fdc])
                    else:
                        _q(in_q).dma_start(out=itv, in_=t_in[i, :, :, c0:c0 + fdc])
                st[ci] = dict(it=it)

            def S0(ci):
                i, c0, fdc = chunks[ci]
                it = st[ci]["it"]
                l = it[:, 0:fdc]
                x = it[:, 2 * fdc:3 * fdc]
                sld = tp.tile([_P, fdc], _F16, tag="sld")
                sgx = tp.tile([_P, fdc], _F16, tag="sgx")
                if hp_act:
                    with tc.high_priority():
                        nc.scalar.activation(sld[:], l, _AF.Tanh, scale=-0.5)
                        nc.scalar.activation(sgx[:], x, _AF.Sigmoid)
                elif act_order == 1:
                    nc.scalar.activation(sgx[:], x, _AF.Sigmoid)
                    nc.scalar.activation(sld[:], l, _AF.Tanh, scale=-0.5)
                elif act_order == 2:
                    pass  # emitted after rx2 below
                else:
                    nc.scalar.activation(sld[:], l, _AF.Tanh, scale=-0.5)
                    nc.scalar.activation(sgx[:], x, _AF.Sigmoid)
                dve_out = (ci < dve_head or ci >= n - dve_tail or
                           (dve_every and ci % dve_every == 0))
                if act_order == 2 and dve_out:
                    nc.scalar.activation(sld[:], l, _AF.Tanh, scale=-0.5)
                    nc.scalar.activation(sgx[:], x, _AF.Sigmoid)
                if not dve_out:
                    h = fdc // 2 if (half_out and fdc == fd) else 0
                    rx2 = tp.tile([_P, fdc - h], _F16, tag="rx2")
                    if (rx_set is not None and ci in rx_set) or \
                            (rx_dve and (ci % rx_dve == 0)):
                        rx = tp.tile([_P, fdc - h], _F16, tag="rx")
                        nc.vector.tensor_scalar(rx[:], x[:, h:fdc], 0.0, None, _OP.max)
                        nc.scalar.activation(rx2[:], rx[:], _AF.Square)
                    else:
                        nc.scalar.activation(rx2[:], x[:, h:fdc], _AF.Relu)
                        if act_order == 2:
                            nc.scalar.activation(sld[:], l, _AF.Tanh, scale=-0.5)
                            nc.scalar.activation(sgx[:], x, _AF.Sigmoid)
                        nc.scalar.activation(rx2[:], rx2[:], _AF.Square)
                    st[ci].update(rx2=rx2, h=h)
                st[ci].update(sld=sld, sgx=sgx, dve_out=dve_out)

            def S1(ci):
                i, c0, fdc = chunks[ci]
                d = st[ci]
                it = d["it"]
                l3 = it[:, 0:fdc].unsqueeze(1)
                u3 = it[:, fdc:2 * fdc].unsqueeze(1)
                sld3 = d["sld"][:].unsqueeze(1)
                ot = otp.tile([_P, 3 * fdc], _F16, tag="out")
                p2 = tp.tile([_P, fdc], _F16, tag="p2")
                sd = tp.tile([_P, fdc], _F16, tag="sd")
                nc.vector._custom_dve(_OP_P2, out=p2[:].unsqueeze(1), in0=l3,
                                      in1=u3, s0=_INV2Z, s1=_FOURZ)
                nc.vector._custom_dve(_OP_SD, out=sd[:].unsqueeze(1), in0=u3,
                                      in1=sld3, s0=_Z16, s1=-_BIG)
                nc.vector._custom_dve(_OP_NU, out=ot[:, 2 * fdc:3 * fdc].unsqueeze(1),
                                      in0=u3, in1=sld3)
                if d["dve_out"]:
                    x3 = it[:, 2 * fdc:3 * fdc].unsqueeze(1)
                    nc.vector._custom_dve(_OP_OUT, out=ot[:, 0:fdc].unsqueeze(1),
                                          in0=x3, in1=d["sgx"][:].unsqueeze(1),
                                          s0=0.5)
                else:
                    h = d["h"]
                    if h:
                        x3h = it[:, 2 * fdc:2 * fdc + h].unsqueeze(1)
                        nc.vector._custom_dve(
                            _OP_OUT, out=ot[:, 0:h].unsqueeze(1), in0=x3h,
                            in1=d["sgx"][:, 0:h].unsqueeze(1), s0=0.5)
                    sg = tp.tile([_P, fdc - h], _F16, tag="sg")
                    eng = nc.vector if sg_dve else nc.gpsimd
                    eng.tensor_scalar(sg[:], d["sgx"][:, h:fdc], 0.5, None,
                                      _OP.min)
                    st[ci].update(sg=sg)
                st[ci].update(ot=ot, p2=p2, sd=sd)

            def S2(ci):
                i, c0, fdc = chunks[ci]
                d = st[ci]
                ot = d["ot"]
                if not d["dve_out"]:
                    h = d["h"]
                    nc.gpsimd.tensor_tensor(ot[:, h:fdc], d["rx2"][:], d["sg"][:],
                                            _OP.subtract)
                if hp_max:
                    with tc.high_priority():
                        nc.vector.tensor_tensor(ot[:, fdc:2 * fdc], d["p2"][:],
                                                d["sd"][:], _OP.max)
                else:
                    nc.vector.tensor_tensor(ot[:, fdc:2 * fdc], d["p2"][:],
                                            d["sd"][:], _OP.max)
                if split_out and not d["dve_out"]:
                    otv = ot[:].rearrange("p (s f) -> p s f", s=3)
                    _q(out_q).dma_start(
                        out=t_out[i, :, 1:3, c0:c0 + fdc],
                        in_=otv[:, 1:3, :])
                    _q(out_q).dma_start(
                        out=t_out[i, :, 0:1, c0:c0 + fdc],
                        in_=otv[:, 0:1, :])
                elif hp_out:
                    with tc.high_priority():
                        _q(out_q).dma_start(
                            out=t_out[i, :, :, c0:c0 + fdc],
                            in_=ot[:].rearrange("p (s f) -> p s f", s=3))
                else:
                    _q(out_q).dma_start(
                        out=t_out[i, :, :, c0:c0 + fdc],
                        in_=ot[:].rearrange("p (s f) -> p s f", s=3))
                st[ci] = None

            if skew == 0:
                for ci in range(n):
                    SDMA(ci); S0(ci); S1(ci); S2(ci)
            elif skew == 1:
                for k in range(n + 1):
                    if k < n: SDMA(k)
                    if 0 <= k - 1 < n:
                        S0(k - 1); S1(k - 1); S2(k - 1)
            elif skew == 2:
                for k in range(n + 2):
                    if k < n: SDMA(k)
                    if 0 <= k - 1 < n: S0(k - 1)
                    if 0 <= k - 2 < n: S1(k - 2); S2(k - 2)
            else:
                order = {
                    "dma_first": lambda k: [(SDMA, k), (S0, k - 1), (S1, k - 2), (S2, k - 3)],
                    "s2_mid": lambda k: [(SDMA, k), (S0, k - 1), (S2, k - 3), (S1, k - 2)],
                }[emit_order]
                for k in range(n + 3):
                    for fn, ci in order(k):
                        if 0 <= ci < n:
                            fn(ci)

    nc.compile()
    return nc


_NC_CACHE = {}


def _get_nc(**kw):
    key = tuple(sorted(kw.items()))
    if key not in _NC_CACHE:
        _NC_CACHE[key] = _build_nc(**kw)
    return _NC_CACHE[key]


def _prep_inputs(x, lower_bounds, upper_bounds):
    """fp16 conversion with case-boundary pinning (see module docstring)."""
    F16 = np.float16
    x16 = x.astype(F16)
    l16 = lower_bounds.astype(F16)
    u16 = upper_bounds.astype(F16)
    # l<0 must stay strictly negative AND large enough that tanh(-l/2)
    # cannot round to zero in fp16 (sld's sign carries [l<0] on device).
    l16 = np.where((lower_bounds < 0) & (l16 >= -2.5e-7), F16(-2.5e-7), l16)
    # u>0 must stay strictly positive (case selection uses u<=0).
    u16 = np.where((upper_bounds > 0) & (u16 <= 0), F16(6e-8), u16)
    # u vs Z: the reference jumps at u==Z; keep each element on its f32 side.
    z16 = F16(_Z16)
    below = np.nextafter(z16, F16(0))
    u16 = np.where((upper_bounds >= _Z32) & (u16 < z16), z16, u16)
    u16 = np.where((upper_bounds < _Z32) & (u16 >= z16), below, u16)
    return x16, l16, u16


def _run(x, lower_bounds, upper_bounds, trace=False, **build_kw):
    assert x.shape == (_N,) and x.dtype == np.float32
    nc = _get_nc(**build_kw)
    fd = build_kw.get("fd", 2048)
    nt = _FDT // fd
    x16, l16, u16 = _prep_inputs(x, lower_bounds, upper_bounds)
    shp = (_NCORES, nt, _P, fd)
    packed = np.empty((_NCORES, nt, _P, 3, fd), dtype=np.float16)
    packed[..., 0, :] = l16.reshape(shp)
    packed[..., 1, :] = u16.reshape(shp)
    packed[..., 2, :] = x16.reshape(shp)
    in_maps = [{"pin": packed[c]} for c in range(_NCORES)]
    res = run_bass_kernel_spmd(
        nc, in_maps, core_ids=list(range(_NCORES)), trace=trace
    )
    pout = np.stack([res.results[c]["pout"] for c in range(_NCORES)])
    out = np.ascontiguousarray(pout[..., 0, :]).reshape(-1).astype(np.float32)
    nl = np.ascontiguousarray(pout[..., 1, :]).reshape(-1).astype(np.float32)
    nl = nl * 0.5 - 0.5  # device emits nl in doubled +0.5 space
    nu = np.ascontiguousarray(pout[..., 2, :]).reshape(-1).astype(np.float32)
    nu = nu * 0.5  # device emits nu in doubled space
    return (out, nl, nu), res


def kernel(x, lower_bounds, upper_bounds):
    (out, nl, nu), _ = _run(x, lower_bounds, upper_bounds)
    return (out, nl, nu)


# revision 31
# speedup vs baseline: 1.0201x; 1.0029x over previous
"""DeepPoly SPU transformer — Trainium2 Bass kernel (custom-DVE edition).

Elementwise over N=16777216; sharded across 8 NeuronCores (2M elems each,
viewed as [nt x 128 x fd] fp16).  All wire traffic is fp16 (24MB/core round
trip = ~69.9us at the 360GB/s DMA roofline); the three input streams are
packed into one DRAM tensor (and the three outputs into another) so each
chunk needs one input DMA (+ a split [nl|nu]/[out] pair of output DMAs so
the early streams can fill DMA gaps).  The whole per-element DAG is
collapsed into 3 fused custom-DVE ops + a DVE tensor-tensor max + 2 ACT
transcendentals + an ACT/Pool tail for the `out` stream, so every engine
sits below the DMA roofline (~8.7us per 128x2048 chunk): per full chunk
ACT 7.6us, DVE 7.7us, Pool 6.9us vs DMA 8.7us.  The first tile is split
into quarter/half chunks (and routed through the low-latency all-DVE out
path) to shorten the pipeline fill; the nl max runs at high priority so
its out-DMA is never stuck behind the next chunk's fused ops.  Measured
74894ns vs the ~73.3us model floor (first-DMA latency + DMA busy + final
sem prop); 73913ns with the joint-sweep config (io_bufs=5, tmp_bufs=2,
high-priority nl-max and out-DMAs — the knobs only win in combination).

Math (per element; Z = sqrt(0.5), spu(t) = t^2-0.5 for t>=0 else
sigmoid(-t)-1).  Device emits nl/nu in doubled space (host applies the
constant affine epilogue  nl = 0.5*nlD - 0.5,  nu = 0.5*nuD):

  sld = tanh(-l/2) = 2*(sigmoid(-l) - 0.5)      [ACT; sign(sld) = -sign(l)]
  sgx = sigmoid(x)                              [ACT]
  out = relu(x)^2 - min(sgx, 0.5)               [ACT relu/square + Pool]
  P2  = 2*g2m*(l - g2m/4),  g2m = 2Z*max([l<0], (u+l)/(2Z))   [custom DVE:
        G1 = max([l<0], (u+l)/(2Z));  P2 = G1*(4Z*l - G1)]
  SD  = sld*[u<=0] - BIG*([u>=Z] + [sld<=0])    [custom DVE]
  nuD = max(2*relu(u)^2, sld) + (2*[sld<=0]-1)  [custom DVE]
  nlD = max(P2, SD)                             [DVE tt max]

Case boundaries (l vs 0, u vs 0, u vs Z) are pinned to the f32 side during
host-side fp16 conversion; l<0 is additionally kept <= -2.5e-7 so that
tanh(-l/2) cannot round to +-0 in fp16 (sld's sign carries [l<0] on
device).  Validated vs the f64 reference: relmax_vs_scale ~ 1e-3 on all
three outputs (tolerance 2e-2).
"""

import numpy as np

import concourse.bass as bass
import concourse.bacc as bacc
import concourse.mybir as mybir
from concourse.tile import TileContext
from concourse.bass_utils import run_bass_kernel_spmd
import concourse.dve_ops as dve_ops
from concourse.dve_spec import (
    Spec, Src0, Src1, C0, C1, Zero, One, maxx, minn, relu, sq, lower as _dve_lower,
    _has_src1,
)
from concourse.dve_uop import DveOpSpec

_N = 16777216
_NCORES = 8
_P = 128
_FDT = _N // _NCORES // _P  # 16384 free elems per partition per core

_Z32 = np.float32(np.sqrt(0.5))
_Z16 = float(np.float16(np.sqrt(0.5)))
_INV2Z = float(np.float32(1.0 / (2.0 * float(_Z32))))
_FOURZ = float(np.float32(4.0 * float(_Z32)))
_BIG = 1000.0

_AF = mybir.ActivationFunctionType
_OP = mybir.AluOpType
_F16 = mybir.dt.float16


# --------------------------------------------------------------------------
# custom DVE ops (registered once per process via the documented extension
# point in concourse.dve_ops; names are namespaced to this kernel)
# --------------------------------------------------------------------------

def _register(name, spec):
    if name in dve_ops._SUB_OPCODE_FOR_NAME:
        for op in dve_ops.OPS:
            if op.name == name:
                return op
    row = max(dve_ops._SUB_OPCODE_FOR_NAME.values()) + 1
    assert row < 0x20, "custom-DVE opcode rows exhausted"
    shas = {}
    for ver in ("v3", "v4"):
        u = _dve_lower(spec, ver=ver)
        shas[ver] = DveOpSpec(
            name=name, opcode=row, uops=u, rd1_en=_has_src1(spec)
        ).sha(ver)
    op = dve_ops.DveOp(name, spec, subdim=False, uops_sha=shas)
    dve_ops.OPS.append(op)
    dve_ops._SUB_OPCODE_FOR_NAME[name] = row
    dve_ops.CUSTOM_DVE_SPECS[name] = spec
    return op


def _ref_p2(in0, in1, c0, c1, c2):
    l = in0.astype(np.float32)
    u = in1.astype(np.float32)
    s2s = (l + u) * np.float32(c0)
    g1 = np.maximum((l < 0).astype(np.float32), s2s)
    return g1 * (l * np.float32(c1) - g1)


_g1 = maxx(Src0 < Zero, (Src0 + Src1) * C0)
_SPEC_P2 = Spec(body=_g1 * (Src0 * C1 - _g1), reference=_ref_p2)


def _ref_sd(in0, in1, c0, c1, c2):
    u = in0.astype(np.float32)
    sld = in1.astype(np.float32)
    m1 = (u <= 0).astype(np.float32)
    m2 = (u >= np.float32(c0)).astype(np.float32)
    m3 = (sld <= 0).astype(np.float32)
    return sld * m1 + (m2 + m3) * np.float32(c1)


_SPEC_SD = Spec(
    body=Src1 * (Src0 <= Zero) + ((Src0 >= C0) + (Src1 <= Zero)) * C1,
    reference=_ref_sd,
)


def _ref_nu(in0, in1, c0, c1, c2):
    u = in0.astype(np.float32)
    sld = in1.astype(np.float32)
    r2 = np.maximum(u, 0) ** 2
    c = (sld <= 0).astype(np.float32)
    return np.maximum(r2 + r2, sld) + (c + c - 1.0)


_r2 = sq(relu(Src0))
_c = (Src1 <= Zero)
_SPEC_NU = Spec(body=maxx(_r2 + _r2, Src1) + (_c + _c - One), reference=_ref_nu)


def _ref_out(in0, in1, c0, c1, c2):
    x = in0.astype(np.float32)
    sgx = in1.astype(np.float32)
    return np.maximum(x, 0) ** 2 - np.minimum(sgx, np.float32(c0))


_SPEC_OUT = Spec(body=sq(relu(Src0)) - minn(Src1, C0), reference=_ref_out)

_OP_P2 = _register("SPU_P2_ANT", _SPEC_P2)
_OP_SD = _register("SPU_SD_ANT", _SPEC_SD)
_OP_NU = _register("SPU_NU_ANT", _SPEC_NU)
_OP_OUT = _register("SPU_OUT_ANT", _SPEC_OUT)


# --------------------------------------------------------------------------
# kernel build
# --------------------------------------------------------------------------

def _build_nc(fd=2048, io_bufs=5, out_bufs=4, tmp_bufs=2, ramp="s24e2",
              dve_head=2, dve_tail=0, rx_dve=0, in_q="sp", out_q="sp",
              emit_order="dma_first", skew=3, hp_out=1, split_out=1, split_in=0,
              half_out=0, hp_act=0, hp_max=1, sg_dve=0, act_order=1, dve_every=0,
              rx_set=None, fdt=_FDT):
    """dve_head/dve_tail: how many chunks at each end of the pipeline route
    their `out` stream through the fused DVE op (low latency) instead of the
    ACT/Pool tail (better steady-state balance).  rx_dve: chunks whose
    relu(x) runs as a DVE tensor_scalar instead of ACT."""
    nc = bacc.Bacc(trn_type="TRN2", debug=False, num_devices=_NCORES)
    nt = fdt // fd

    def _q(name):
        return {"sp": nc.sync, "act": nc.scalar, "dve": nc.vector,
                "pool": nc.gpsimd}[name]

    t_in = nc.dram_tensor("pin", [nt, _P, 3, fd], _F16, kind="ExternalInput")
    t_out = nc.dram_tensor("pout", [nt, _P, 3, fd], _F16, kind="ExternalOutput")

    with TileContext(nc) as tc:
        with tc.tile_pool(name="io", bufs=io_bufs) as iop, \
             tc.tile_pool(name="ot", bufs=out_bufs) as otp, \
             tc.tile_pool(name="tmp", bufs=tmp_bufs) as tp:

            if ramp == "end":
                chunks = [(i, 0, fd) for i in range(nt - 1)]
                chunks += [(nt - 1, c, fd // 2) for c in range(0, fd, fd // 2)]
            elif ramp == "end4":
                chunks = [(i, 0, fd) for i in range(nt - 1)]
                chunks += [(nt - 1, c, fd // 4) for c in range(0, fd, fd // 4)]
            elif ramp == "both":
                chunks = [(0, c, fd // 2) for c in range(0, fd, fd // 2)]
                chunks += [(i, 0, fd) for i in range(1, nt - 1)]
                chunks += [(nt - 1, c, fd // 4) for c in range(0, fd, fd // 4)]
            elif ramp == "s4e2":
                chunks = [(0, c, fd // 4) for c in range(0, fd, fd // 4)]
                chunks += [(i, 0, fd) for i in range(1, nt - 1)]
                chunks += [(nt - 1, c, fd // 2) for c in range(0, fd, fd // 2)]
            elif ramp == "s4e4":
                chunks = [(0, c, fd // 4) for c in range(0, fd, fd // 4)]
                chunks += [(i, 0, fd) for i in range(1, nt - 1)]
                chunks += [(nt - 1, c, fd // 4) for c in range(0, fd, fd // 4)]
            elif ramp == "s2e2":
                chunks = [(0, c, fd // 2) for c in range(0, fd, fd // 2)]
                chunks += [(i, 0, fd) for i in range(1, nt - 1)]
                chunks += [(nt - 1, c, fd // 2) for c in range(0, fd, fd // 2)]
            elif ramp == "s24e2":
                chunks = [(0, 0, fd // 4), (0, fd // 4, fd // 4), (0, fd // 2, fd // 2)]
                chunks += [(i, 0, fd) for i in range(1, nt - 1)]
                chunks += [(nt - 1, c, fd // 2) for c in range(0, fd, fd // 2)]
            elif ramp == "s24e24":
                chunks = [(0, 0, fd // 4), (0, fd // 4, fd // 4), (0, fd // 2, fd // 2)]
                chunks += [(i, 0, fd) for i in range(1, nt - 1)]
                chunks += [(nt - 1, 0, fd // 2), (nt - 1, fd // 2, fd // 4),
                           (nt - 1, 3 * fd // 4, fd // 4)]
            elif ramp.startswith("m"):
                # s24 start, split tile <m> (and optionally more) into halves
                msp = {int(c) for c in ramp[1:].split("_")}
                chunks = [(0, 0, fd // 4), (0, fd // 4, fd // 4), (0, fd // 2, fd // 2)]
                for i in range(1, nt - 1):
                    if i in msp:
                        chunks += [(i, 0, fd // 2), (i, fd // 2, fd // 2)]
                    else:
                        chunks.append((i, 0, fd))
                chunks += [(nt - 1, c, fd // 2) for c in range(0, fd, fd // 2)]
            elif ramp == "s42e2":
                chunks = [(0, 0, fd // 2), (0, fd // 2, fd // 4),
                          (0, 3 * fd // 4, fd // 4)]
                chunks += [(i, 0, fd) for i in range(1, nt - 1)]
                chunks += [(nt - 1, c, fd // 2) for c in range(0, fd, fd // 2)]
            elif ramp == "s124e2":
                chunks = [(0, 0, fd // 8), (0, fd // 8, fd // 8),
                          (0, fd // 4, fd // 4), (0, fd // 2, fd // 2)]
                chunks += [(i, 0, fd) for i in range(1, nt - 1)]
                chunks += [(nt - 1, c, fd // 2) for c in range(0, fd, fd // 2)]
            elif ramp == "s24":
                chunks = [(0, 0, fd // 4), (0, fd // 4, fd // 4), (0, fd // 2, fd // 2)]
                chunks += [(i, 0, fd) for i in range(1, nt)]
            elif ramp == "s24e2x":
                chunks = [(0, 0, fd // 4), (0, fd // 4, fd // 4), (0, fd // 2, fd // 2)]
                chunks += [(i, 0, fd) for i in range(1, nt - 2)]
                chunks += [(i, c, fd // 2) for i in (nt - 2, nt - 1)
                           for c in range(0, fd, fd // 2)]
            elif ramp == "s8e2":
                chunks = [(0, c, fd // 8) for c in range(0, fd, fd // 8)]
                chunks += [(i, 0, fd) for i in range(1, nt - 1)]
                chunks += [(nt - 1, c, fd // 2) for c in range(0, fd, fd // 2)]
            else:
                chunks = [(i, 0, fd) for i in range(nt)]

            n = len(chunks)
            st = [None] * n

            def SDMA(ci):
                i, c0, fdc = chunks[ci]
                with tc.high_priority():
                    it = iop.tile([_P, 3 * fdc], _F16, tag="in")
                    itv = it[:].rearrange("p (s f) -> p s f", s=3)
                    if split_in:
                        _q(in_q).dma_start(out=itv[:, 0:2, :],
                                           in_=t_in[i, :, 0:2, c0:c0 + fdc])
                        _q(in_q).dma_start(out=itv[:, 2:3, :],
                                           in_=t_in[i, :, 2:3, c0:c0 + fdc])
                    else:
                        _q(in_q).dma_start(out=itv, in_=t_in[i, :, :, c0:c0 + fdc])
                st[ci] = dict(it=it)

            def S0(ci):
                i, c0, fdc = chunks[ci]
                it = st[ci]["it"]
                l = it[:, 0:fdc]
                x = it[:, 2 * fdc:3 * fdc]
                sld = tp.tile([_P, fdc], _F16, tag="sld")
                sgx = tp.tile([_P, fdc], _F16, tag="sgx")
                if hp_act:
                    with tc.high_priority():
                        nc.scalar.activation(sld[:], l, _AF.Tanh, scale=-0.5)
                        nc.scalar.activation(sgx[:], x, _AF.Sigmoid)
                elif act_order == 1:
                    nc.scalar.activation(sgx[:], x, _AF.Sigmoid)
                    nc.scalar.activation(sld[:], l, _AF.Tanh, scale=-0.5)
                elif act_order == 2:
                    pass  # emitted after rx2 below
                else:
                    nc.scalar.activation(sld[:], l, _AF.Tanh, scale=-0.5)
                    nc.scalar.activation(sgx[:], x, _AF.Sigmoid)
                dve_out = (ci < dve_head or ci >= n - dve_tail or
                           (dve_every and ci % dve_every == 0))
                if act_order == 2 and dve_out:
                    nc.scalar.activation(sld[:], l, _AF.Tanh, scale=-0.5)
                    nc.scalar.activation(sgx[:], x, _AF.Sigmoid)
                if not dve_out:
                    h = fdc // 2 if (half_out and fdc == fd) else 0
                    rx2 = tp.tile([_P, fdc - h], _F16, tag="rx2")
                    if (rx_set is not None and ci in rx_set) or \
                            (rx_dve and (ci % rx_dve == 0)):
                        rx = tp.tile([_P, fdc - h], _F16, tag="rx")
                        nc.vector.tensor_scalar(rx[:], x[:, h:fdc], 0.0, None, _OP.max)
                        nc.scalar.activation(rx2[:], rx[:], _AF.Square)
                    else:
                        nc.scalar.activation(rx2[:], x[:, h:fdc], _AF.Relu)
                        if act_order == 2:
                            nc.scalar.activation(sld[:], l, _AF.Tanh, scale=-0.5)
                            nc.scalar.activation(sgx[:], x, _AF.Sigmoid)
                        nc.scalar.activation(rx2[:], rx2[:], _AF.Square)
                    st[ci].update(rx2=rx2, h=h)
                st[ci].update(sld=sld, sgx=sgx, dve_out=dve_out)

            def S1(ci):
                i, c0, fdc = chunks[ci]
                d = st[ci]
                it = d["it"]
                l3 = it[:, 0:fdc].unsqueeze(1)
                u3 = it[:, fdc:2 * fdc].unsqueeze(1)
                sld3 = d["sld"][:].unsqueeze(1)
                ot = otp.tile([_P, 3 * fdc], _F16, tag="out")
                p2 = tp.tile([_P, fdc], _F16, tag="p2")
                sd = tp.tile([_P, fdc], _F16, tag="sd")
                nc.vector._custom_dve(_OP_P2, out=p2[:].unsqueeze(1), in0=l3,
                                      in1=u3, s0=_INV2Z, s1=_FOURZ)
                nc.vector._custom_dve(_OP_SD, out=sd[:].unsqueeze(1), in0=u3,
                                      in1=sld3, s0=_Z16, s1=-_BIG)
                nc.vector._custom_dve(_OP_NU, out=ot[:, 2 * fdc:3 * fdc].unsqueeze(1),
                                      in0=u3, in1=sld3)
                if d["dve_out"]:
                    x3 = it[:, 2 * fdc:3 * fdc].unsqueeze(1)
                    nc.vector._custom_dve(_OP_OUT, out=ot[:, 0:fdc].unsqueeze(1),
                                          in0=x3, in1=d["sgx"][:].unsqueeze(1),
                                          s0=0.5)
                else:
                    h = d["h"]
                    if h:
                        x3h = it[:, 2 * fdc:2 * fdc + h].unsqueeze(1)
                        nc.vector._custom_dve(
                            _OP_OUT, out=ot[:, 0:h].unsqueeze(1), in0=x3h,
                            in1=d["sgx"][:, 0:h].unsqueeze(1), s0=0.5)
                    sg = tp.tile([_P, fdc - h], _F16, tag="sg")
                    eng = nc.vector if sg_dve else nc.gpsimd
                    eng.tensor_scalar(sg[:], d["sgx"][:, h:fdc], 0.5, None,
                                      _OP.min)
                    st[ci].update(sg=sg)
                st[ci].update(ot=ot, p2=p2, sd=sd)

            def S2(ci):
                i, c0, fdc = chunks[ci]
                d = st[ci]
                ot = d["ot"]
                if not d["dve_out"]:
                    h = d["h"]
                    nc.gpsimd.tensor_tensor(ot[:, h:fdc], d["rx2"][:], d["sg"][:],
                                            _OP.subtract)
                if hp_max:
                    with tc.high_priority():
                        nc.vector.tensor_tensor(ot[:, fdc:2 * fdc], d["p2"][:],
                                                d["sd"][:], _OP.max)
                else:
                    nc.vector.tensor_tensor(ot[:, fdc:2 * fdc], d["p2"][:],
                                            d["sd"][:], _OP.max)
                if split_out and not d["dve_out"]:
                    otv = ot[:].rearrange("p (s f) -> p s f", s=3)
                    _q(out_q).dma_start(
                        out=t_out[i, :, 1:3, c0:c0 + fdc],
                        in_=otv[:, 1:3, :])
                    _q(out_q).dma_start(
                        out=t_out[i, :, 0:1, c0:c0 + fdc],
                        in_=otv[:, 0:1, :])
                elif hp_out:
                    with tc.high_priority():
                        _q(out_q).dma_start(
                            out=t_out[i, :, :, c0:c0 + fdc],
                            in_=ot[:].rearrange("p (s f) -> p s f", s=3))
                else:
                    _q(out_q).dma_start(
                        out=t_out[i, :, :, c0:c0 + fdc],
                        in_=ot[:].rearrange("p (s f) -> p s f", s=3))
                st[ci] = None

            if skew == 0:
                for ci in range(n):
                    SDMA(ci); S0(ci); S1(ci); S2(ci)
            elif skew == 1:
                for k in range(n + 1):
                    if k < n: SDMA(k)
                    if 0 <= k - 1 < n:
                        S0(k - 1); S1(k - 1); S2(k - 1)
            elif skew == 2:
                for k in range(n + 2):
                    if k < n: SDMA(k)
                    if 0 <= k - 1 < n: S0(k - 1)
                    if 0 <= k - 2 < n: S1(k - 2); S2(k - 2)
            else:
                order = {
                    "dma_first": lambda k: [(SDMA, k), (S0, k - 1), (S1, k - 2), (S2, k - 3)],
                    "s2_mid": lambda k: [(SDMA, k), (S0, k - 1), (S2, k - 3), (S1, k - 2)],
                }[emit_order]
                for k in range(n + 3):
                    for fn, ci in order(k):
                        if 0 <= ci < n:
                            fn(ci)

    nc.compile()
    return nc


_NC_CACHE = {}


def _get_nc(**kw):
    key = tuple(sorted(kw.items()))
    if key not in _NC_CACHE:
        _NC_CACHE[key] = _build_nc(**kw)
    return _NC_CACHE[key]


def _prep_inputs(x, lower_bounds, upper_bounds):
    """fp16 conversion with case-boundary pinning (see module docstring)."""
    F16 = np.float16
    x16 = x.astype(F16)
    l16 = lower_bounds.astype(F16)
    u16 = upper_bounds.astype(F16)
    # l<0 must stay strictly negative AND large enough that tanh(-l/2)
    # cannot round to zero in fp16 (sld's sign carries [l<0] on device).
    l16 = np.where((lower_bounds < 0) & (l16 >= -2.5e-7), F16(-2.5e-7), l16)
    # u>0 must stay strictly positive (case selection uses u<=0).
    u16 = np.where((upper_bounds > 0) & (u16 <= 0), F16(6e-8), u16)
    # u vs Z: the reference jumps at u==Z; keep each element on its f32 side.
    z16 = F16(_Z16)
    below = np.nextafter(z16, F16(0))
    u16 = np.where((upper_bounds >= _Z32) & (u16 < z16), z16, u16)
    u16 = np.where((upper_bounds < _Z32) & (u16 >= z16), below, u16)
    return x16, l16, u16


def _run(x, lower_bounds, upper_bounds, trace=False, **build_kw):
    assert x.shape == (_N,) and x.dtype == np.float32
    nc = _get_nc(**build_kw)
    fd = build_kw.get("fd", 2048)
    nt = _FDT // fd
    x16, l16, u16 = _prep_inputs(x, lower_bounds, upper_bounds)
    shp = (_NCORES, nt, _P, fd)
    packed = np.empty((_NCORES, nt, _P, 3, fd), dtype=np.float16)
    packed[..., 0, :] = l16.reshape(shp)
    packed[..., 1, :] = u16.reshape(shp)
    packed[..., 2, :] = x16.reshape(shp)
    in_maps = [{"pin": packed[c]} for c in range(_NCORES)]
    res = run_bass_kernel_spmd(
        nc, in_maps, core_ids=list(range(_NCORES)), trace=trace
    )
    pout = np.stack([res.results[c]["pout"] for c in range(_NCORES)])
    out = np.ascontiguousarray(pout[..., 0, :]).reshape(-1).astype(np.float32)
    nl = np.ascontiguousarray(pout[..., 1, :]).reshape(-1).astype(np.float32)
    nl = nl * 0.5 - 0.5  # device emits nl in doubled +0.5 space
    nu = np.ascontiguousarray(pout[..., 2, :]).reshape(-1).astype(np.float32)
    nu = nu * 0.5  # device emits nu in doubled space
    return (out, nl, nu), res


def kernel(x, lower_bounds, upper_bounds):
    (out, nl, nu), _ = _run(x, lower_bounds, upper_bounds)
    return (out, nl, nu)


# revision 33
# speedup vs baseline: 1.0249x; 1.0046x over previous
"""DeepPoly SPU transformer — Trainium2 Bass kernel (custom-DVE edition).

Elementwise over N=16777216; sharded across 8 NeuronCores (2M elems each,
viewed as [nt x 128 x fd] fp16).  All wire traffic is fp16 (24MB/core round
trip = ~69.9us at the 360GB/s DMA roofline); the three input streams are
packed into one DRAM tensor (and the three outputs into another) so each
chunk needs one input DMA (+ a split [nl|nu]/[out] pair of output DMAs so
the early streams can fill DMA gaps).  The whole per-element DAG is
collapsed into 3 fused custom-DVE ops + a DVE tensor-tensor max + 2 ACT
transcendentals + an ACT/Pool tail for the `out` stream, so every engine
sits below the DMA roofline (~8.7us per 128x2048 chunk): per full chunk
ACT 7.6us, DVE 7.7us, Pool 6.9us vs DMA 8.7us.  The first tile is split
into quarter/half chunks (and routed through the low-latency all-DVE out
path) to shorten the pipeline fill; the nl max runs at high priority so
its out-DMA is never stuck behind the next chunk's fused ops.  Measured
74894ns vs the ~73.3us model floor (first-DMA latency + DMA busy + final
sem prop); 73702ns with the joint-sweep config (io_bufs=5, tmp_bufs=2,
high-priority nl-max and out-DMAs, sigmoid-first ACT order — the knobs only win jointly).

Math (per element; Z = sqrt(0.5), spu(t) = t^2-0.5 for t>=0 else
sigmoid(-t)-1).  Device emits nl/nu in doubled space (host applies the
constant affine epilogue  nl = 0.5*nlD - 0.5,  nu = 0.5*nuD):

  sld = tanh(-l/2) = 2*(sigmoid(-l) - 0.5)      [ACT; sign(sld) = -sign(l)]
  sgx = sigmoid(x)                              [ACT]
  out = relu(x)^2 - min(sgx, 0.5)               [ACT relu/square + Pool]
  P2  = 2*g2m*(l - g2m/4),  g2m = 2Z*max([l<0], (u+l)/(2Z))   [custom DVE:
        G1 = max([l<0], (u+l)/(2Z));  P2 = G1*(4Z*l - G1)]
  SD  = sld*[u<=0] - BIG*([u>=Z] + [sld<=0])    [custom DVE]
  nuD = max(2*relu(u)^2, sld) + (2*[sld<=0]-1)  [custom DVE]
  nlD = max(P2, SD)                             [DVE tt max]

Case boundaries (l vs 0, u vs 0, u vs Z) are pinned to the f32 side during
host-side fp16 conversion; l<0 is additionally kept <= -2.5e-7 so that
tanh(-l/2) cannot round to +-0 in fp16 (sld's sign carries [l<0] on
device).  Validated vs the f64 reference: relmax_vs_scale ~ 1e-3 on all
three outputs (tolerance 2e-2).
"""

import numpy as np

import concourse.bass as bass
import concourse.bacc as bacc
import concourse.mybir as mybir
from concourse.tile import TileContext
from concourse.bass_utils import run_bass_kernel_spmd
import concourse.dve_ops as dve_ops
from concourse.dve_spec import (
    Spec, Src0, Src1, C0, C1, Zero, One, maxx, minn, relu, sq, lower as _dve_lower,
    _has_src1,
)
from concourse.dve_uop import DveOpSpec

_N = 16777216
_NCORES = 8
_P = 128
_FDT = _N // _NCORES // _P  # 16384 free elems per partition per core

_Z32 = np.float32(np.sqrt(0.5))
_Z16 = float(np.float16(np.sqrt(0.5)))
_INV2Z = float(np.float32(1.0 / (2.0 * float(_Z32))))
_FOURZ = float(np.float32(4.0 * float(_Z32)))
_BIG = 1000.0

_AF = mybir.ActivationFunctionType
_OP = mybir.AluOpType
_F16 = mybir.dt.float16


# --------------------------------------------------------------------------
# custom DVE ops (registered once per process via the documented extension
# point in concourse.dve_ops; names are namespaced to this kernel)
# --------------------------------------------------------------------------

def _register(name, spec):
    if name in dve_ops._SUB_OPCODE_FOR_NAME:
        for op in dve_ops.OPS:
            if op.name == name:
                return op
    row = max(dve_ops._SUB_OPCODE_FOR_NAME.values()) + 1
    assert row < 0x20, "custom-DVE opcode rows exhausted"
    shas = {}
    for ver in ("v3", "v4"):
        u = _dve_lower(spec, ver=ver)
        shas[ver] = DveOpSpec(
            name=name, opcode=row, uops=u, rd1_en=_has_src1(spec)
        ).sha(ver)
    op = dve_ops.DveOp(name, spec, subdim=False, uops_sha=shas)
    dve_ops.OPS.append(op)
    dve_ops._SUB_OPCODE_FOR_NAME[name] = row
    dve_ops.CUSTOM_DVE_SPECS[name] = spec
    return op


def _ref_p2(in0, in1, c0, c1, c2):
    l = in0.astype(np.float32)
    u = in1.astype(np.float32)
    s2s = (l + u) * np.float32(c0)
    g1 = np.maximum((l < 0).astype(np.float32), s2s)
    return g1 * (l * np.float32(c1) - g1)


_g1 = maxx(Src0 < Zero, (Src0 + Src1) * C0)
_SPEC_P2 = Spec(body=_g1 * (Src0 * C1 - _g1), reference=_ref_p2)


def _ref_sd(in0, in1, c0, c1, c2):
    u = in0.astype(np.float32)
    sld = in1.astype(np.float32)
    m1 = (u <= 0).astype(np.float32)
    m2 = (u >= np.float32(c0)).astype(np.float32)
    m3 = (sld <= 0).astype(np.float32)
    return sld * m1 + (m2 + m3) * np.float32(c1)


_SPEC_SD = Spec(
    body=Src1 * (Src0 <= Zero) + ((Src0 >= C0) + (Src1 <= Zero)) * C1,
    reference=_ref_sd,
)


def _ref_nu(in0, in1, c0, c1, c2):
    u = in0.astype(np.float32)
    sld = in1.astype(np.float32)
    r2 = np.maximum(u, 0) ** 2
    c = (sld <= 0).astype(np.float32)
    return np.maximum(r2 + r2, sld) + (c + c - 1.0)


_r2 = sq(relu(Src0))
_c = (Src1 <= Zero)
_SPEC_NU = Spec(body=maxx(_r2 + _r2, Src1) + (_c + _c - One), reference=_ref_nu)


def _ref_out(in0, in1, c0, c1, c2):
    x = in0.astype(np.float32)
    sgx = in1.astype(np.float32)
    return np.maximum(x, 0) ** 2 - np.minimum(sgx, np.float32(c0))


_SPEC_OUT = Spec(body=sq(relu(Src0)) - minn(Src1, C0), reference=_ref_out)

_OP_P2 = _register("SPU_P2_ANT", _SPEC_P2)
_OP_SD = _register("SPU_SD_ANT", _SPEC_SD)
_OP_NU = _register("SPU_NU_ANT", _SPEC_NU)
_OP_OUT = _register("SPU_OUT_ANT", _SPEC_OUT)


# --------------------------------------------------------------------------
# kernel build
# --------------------------------------------------------------------------

def _build_nc(fd=2048, io_bufs=6, out_bufs=5, tmp_bufs=2, ramp="s24e2",
              dve_head=2, dve_tail=0, rx_dve=0, in_q="sp", out_q="sp",
              emit_order="dma_first", skew=3, hp_out=1, split_out=1, split_in=0,
              half_out=0, hp_act=0, hp_max=1, sg_dve=0, act_order=1, dve_every=0,
              rx_set=None, fdt=_FDT):
    """dve_head/dve_tail: how many chunks at each end of the pipeline route
    their `out` stream through the fused DVE op (low latency) instead of the
    ACT/Pool tail (better steady-state balance).  rx_dve: chunks whose
    relu(x) runs as a DVE tensor_scalar instead of ACT."""
    nc = bacc.Bacc(trn_type="TRN2", debug=False, num_devices=_NCORES)
    nt = fdt // fd

    def _q(name):
        return {"sp": nc.sync, "act": nc.scalar, "dve": nc.vector,
                "pool": nc.gpsimd}[name]

    t_in = nc.dram_tensor("pin", [nt, _P, 3, fd], _F16, kind="ExternalInput")
    t_out = nc.dram_tensor("pout", [nt, _P, 3, fd], _F16, kind="ExternalOutput")

    with TileContext(nc) as tc:
        with tc.tile_pool(name="io", bufs=io_bufs) as iop, \
             tc.tile_pool(name="ot", bufs=out_bufs) as otp, \
             tc.tile_pool(name="tmp", bufs=tmp_bufs) as tp:

            if ramp == "end":
                chunks = [(i, 0, fd) for i in range(nt - 1)]
                chunks += [(nt - 1, c, fd // 2) for c in range(0, fd, fd // 2)]
            elif ramp == "end4":
                chunks = [(i, 0, fd) for i in range(nt - 1)]
                chunks += [(nt - 1, c, fd // 4) for c in range(0, fd, fd // 4)]
            elif ramp == "both":
                chunks = [(0, c, fd // 2) for c in range(0, fd, fd // 2)]
                chunks += [(i, 0, fd) for i in range(1, nt - 1)]
                chunks += [(nt - 1, c, fd // 4) for c in range(0, fd, fd // 4)]
            elif ramp == "s4e2":
                chunks = [(0, c, fd // 4) for c in range(0, fd, fd // 4)]
                chunks += [(i, 0, fd) for i in range(1, nt - 1)]
                chunks += [(nt - 1, c, fd // 2) for c in range(0, fd, fd // 2)]
            elif ramp == "s4e4":
                chunks = [(0, c, fd // 4) for c in range(0, fd, fd // 4)]
                chunks += [(i, 0, fd) for i in range(1, nt - 1)]
                chunks += [(nt - 1, c, fd // 4) for c in range(0, fd, fd // 4)]
            elif ramp == "s2e2":
                chunks = [(0, c, fd // 2) for c in range(0, fd, fd // 2)]
                chunks += [(i, 0, fd) for i in range(1, nt - 1)]
                chunks += [(nt - 1, c, fd // 2) for c in range(0, fd, fd // 2)]
            elif ramp == "s24e2":
                chunks = [(0, 0, fd // 4), (0, fd // 4, fd // 4), (0, fd // 2, fd // 2)]
                chunks += [(i, 0, fd) for i in range(1, nt - 1)]
                chunks += [(nt - 1, c, fd // 2) for c in range(0, fd, fd // 2)]
            elif ramp == "s24e24":
                chunks = [(0, 0, fd // 4), (0, fd // 4, fd // 4), (0, fd // 2, fd // 2)]
                chunks += [(i, 0, fd) for i in range(1, nt - 1)]
                chunks += [(nt - 1, 0, fd // 2), (nt - 1, fd // 2, fd // 4),
                           (nt - 1, 3 * fd // 4, fd // 4)]
            elif ramp.startswith("m"):
                # s24 start, split tile <m> (and optionally more) into halves
                msp = {int(c) for c in ramp[1:].split("_")}
                chunks = [(0, 0, fd // 4), (0, fd // 4, fd // 4), (0, fd // 2, fd // 2)]
                for i in range(1, nt - 1):
                    if i in msp:
                        chunks += [(i, 0, fd // 2), (i, fd // 2, fd // 2)]
                    else:
                        chunks.append((i, 0, fd))
                chunks += [(nt - 1, c, fd // 2) for c in range(0, fd, fd // 2)]
            elif ramp == "s42e2":
                chunks = [(0, 0, fd // 2), (0, fd // 2, fd // 4),
                          (0, 3 * fd // 4, fd // 4)]
                chunks += [(i, 0, fd) for i in range(1, nt - 1)]
                chunks += [(nt - 1, c, fd // 2) for c in range(0, fd, fd // 2)]
            elif ramp == "s124e2":
                chunks = [(0, 0, fd // 8), (0, fd // 8, fd // 8),
                          (0, fd // 4, fd // 4), (0, fd // 2, fd // 2)]
                chunks += [(i, 0, fd) for i in range(1, nt - 1)]
                chunks += [(nt - 1, c, fd // 2) for c in range(0, fd, fd // 2)]
            elif ramp == "s24":
                chunks = [(0, 0, fd // 4), (0, fd // 4, fd // 4), (0, fd // 2, fd // 2)]
                chunks += [(i, 0, fd) for i in range(1, nt)]
            elif ramp == "s24e2x":
                chunks = [(0, 0, fd // 4), (0, fd // 4, fd // 4), (0, fd // 2, fd // 2)]
                chunks += [(i, 0, fd) for i in range(1, nt - 2)]
                chunks += [(i, c, fd // 2) for i in (nt - 2, nt - 1)
                           for c in range(0, fd, fd // 2)]
            elif ramp == "s8e2":
                chunks = [(0, c, fd // 8) for c in range(0, fd, fd // 8)]
                chunks += [(i, 0, fd) for i in range(1, nt - 1)]
                chunks += [(nt - 1, c, fd // 2) for c in range(0, fd, fd // 2)]
            else:
                chunks = [(i, 0, fd) for i in range(nt)]

            n = len(chunks)
            st = [None] * n

            def SDMA(ci):
                i, c0, fdc = chunks[ci]
                with tc.high_priority():
                    it = iop.tile([_P, 3 * fdc], _F16, tag="in")
                    itv = it[:].rearrange("p (s f) -> p s f", s=3)
                    if split_in:
                        _q(in_q).dma_start(out=itv[:, 0:2, :],
                                           in_=t_in[i, :, 0:2, c0:c0 + fdc])
                        _q(in_q).dma_start(out=itv[:, 2:3, :],
                                           in_=t_in[i, :, 2:3, c0:c0 + fdc])
                    else:
                        _q(in_q).dma_start(out=itv, in_=t_in[i, :, :, c0:c0 + fdc])
                st[ci] = dict(it=it)

            def S0(ci):
                i, c0, fdc = chunks[ci]
                it = st[ci]["it"]
                l = it[:, 0:fdc]
                x = it[:, 2 * fdc:3 * fdc]
                sld = tp.tile([_P, fdc], _F16, tag="sld")
                sgx = tp.tile([_P, fdc], _F16, tag="sgx")
                if hp_act:
                    with tc.high_priority():
                        nc.scalar.activation(sld[:], l, _AF.Tanh, scale=-0.5)
                        nc.scalar.activation(sgx[:], x, _AF.Sigmoid)
                elif act_order == 1:
                    nc.scalar.activation(sgx[:], x, _AF.Sigmoid)
                    nc.scalar.activation(sld[:], l, _AF.Tanh, scale=-0.5)
                elif act_order == 2:
                    pass  # emitted after rx2 below
                else:
                    nc.scalar.activation(sld[:], l, _AF.Tanh, scale=-0.5)
                    nc.scalar.activation(sgx[:], x, _AF.Sigmoid)
                dve_out = (ci < dve_head or ci >= n - dve_tail or
                           (dve_every and ci % dve_every == 0))
                if act_order == 2 and dve_out:
                    nc.scalar.activation(sld[:], l, _AF.Tanh, scale=-0.5)
                    nc.scalar.activation(sgx[:], x, _AF.Sigmoid)
                if not dve_out:
                    h = fdc // 2 if (half_out and fdc == fd) else 0
                    rx2 = tp.tile([_P, fdc - h], _F16, tag="rx2")
                    if (rx_set is not None and ci in rx_set) or \
                            (rx_dve and (ci % rx_dve == 0)):
                        rx = tp.tile([_P, fdc - h], _F16, tag="rx")
                        nc.vector.tensor_scalar(rx[:], x[:, h:fdc], 0.0, None, _OP.max)
                        nc.scalar.activation(rx2[:], rx[:], _AF.Square)
                    else:
                        nc.scalar.activation(rx2[:], x[:, h:fdc], _AF.Relu)
                        if act_order == 2:
                            nc.scalar.activation(sld[:], l, _AF.Tanh, scale=-0.5)
                            nc.scalar.activation(sgx[:], x, _AF.Sigmoid)
                        nc.scalar.activation(rx2[:], rx2[:], _AF.Square)
                    st[ci].update(rx2=rx2, h=h)
                st[ci].update(sld=sld, sgx=sgx, dve_out=dve_out)

            def S1(ci):
                i, c0, fdc = chunks[ci]
                d = st[ci]
                it = d["it"]
                l3 = it[:, 0:fdc].unsqueeze(1)
                u3 = it[:, fdc:2 * fdc].unsqueeze(1)
                sld3 = d["sld"][:].unsqueeze(1)
                ot = otp.tile([_P, 3 * fdc], _F16, tag="out")
                p2 = tp.tile([_P, fdc], _F16, tag="p2")
                sd = tp.tile([_P, fdc], _F16, tag="sd")
                nc.vector._custom_dve(_OP_P2, out=p2[:].unsqueeze(1), in0=l3,
                                      in1=u3, s0=_INV2Z, s1=_FOURZ)
                nc.vector._custom_dve(_OP_SD, out=sd[:].unsqueeze(1), in0=u3,
                                      in1=sld3, s0=_Z16, s1=-_BIG)
                nc.vector._custom_dve(_OP_NU, out=ot[:, 2 * fdc:3 * fdc].unsqueeze(1),
                                      in0=u3, in1=sld3)
                if d["dve_out"]:
                    x3 = it[:, 2 * fdc:3 * fdc].unsqueeze(1)
                    nc.vector._custom_dve(_OP_OUT, out=ot[:, 0:fdc].unsqueeze(1),
                                          in0=x3, in1=d["sgx"][:].unsqueeze(1),
                                          s0=0.5)
                else:
                    h = d["h"]
                    if h:
                        x3h = it[:, 2 * fdc:2 * fdc + h].unsqueeze(1)
                        nc.vector._custom_dve(
                            _OP_OUT, out=ot[:, 0:h].unsqueeze(1), in0=x3h,
                            in1=d["sgx"][:, 0:h].unsqueeze(1), s0=0.5)
                    sg = tp.tile([_P, fdc - h], _F16, tag="sg")
                    eng = nc.vector if sg_dve else nc.gpsimd
                    eng.tensor_scalar(sg[:], d["sgx"][:, h:fdc], 0.5, None,
                                      _OP.min)
                    st[ci].update(sg=sg)
                st[ci].update(ot=ot, p2=p2, sd=sd)

            def S2(ci):
                i, c0, fdc = chunks[ci]
                d = st[ci]
                ot = d["ot"]
                if not d["dve_out"]:
                    h = d["h"]
                    nc.gpsimd.tensor_tensor(ot[:, h:fdc], d["rx2"][:], d["sg"][:],
                                            _OP.subtract)
                if hp_max:
                    with tc.high_priority():
                        nc.vector.tensor_tensor(ot[:, fdc:2 * fdc], d["p2"][:],
                                                d["sd"][:], _OP.max)
                else:
                    nc.vector.tensor_tensor(ot[:, fdc:2 * fdc], d["p2"][:],
                                            d["sd"][:], _OP.max)
                if split_out and not d["dve_out"]:
                    otv = ot[:].rearrange("p (s f) -> p s f", s=3)
                    _q(out_q).dma_start(
                        out=t_out[i, :, 1:3, c0:c0 + fdc],
                        in_=otv[:, 1:3, :])
                    _q(out_q).dma_start(
                        out=t_out[i, :, 0:1, c0:c0 + fdc],
                        in_=otv[:, 0:1, :])
                elif hp_out:
                    with tc.high_priority():
                        _q(out_q).dma_start(
                            out=t_out[i, :, :, c0:c0 + fdc],
                            in_=ot[:].rearrange("p (s f) -> p s f", s=3))
                else:
                    _q(out_q).dma_start(
                        out=t_out[i, :, :, c0:c0 + fdc],
                        in_=ot[:].rearrange("p (s f) -> p s f", s=3))
                st[ci] = None

            if skew == 0:
                for ci in range(n):
                    SDMA(ci); S0(ci); S1(ci); S2(ci)
            elif skew == 1:
                for k in range(n + 1):
                    if k < n: SDMA(k)
                    if 0 <= k - 1 < n:
                        S0(k - 1); S1(k - 1); S2(k - 1)
            elif skew == 2:
                for k in range(n + 2):
                    if k < n: SDMA(k)
                    if 0 <= k - 1 < n: S0(k - 1)
                    if 0 <= k - 2 < n: S1(k - 2); S2(k - 2)
            else:
                order = {
                    "dma_first": lambda k: [(SDMA, k), (S0, k - 1), (S1, k - 2), (S2, k - 3)],
                    "s2_mid": lambda k: [(SDMA, k), (S0, k - 1), (S2, k - 3), (S1, k - 2)],
                }[emit_order]
                for k in range(n + 3):
                    for fn, ci in order(k):
                        if 0 <= ci < n:
                            fn(ci)

    nc.compile()
    return nc


_NC_CACHE = {}


def _get_nc(**kw):
    key = tuple(sorted(kw.items()))
    if key not in _NC_CACHE:
        _NC_CACHE[key] = _build_nc(**kw)
    return _NC_CACHE[key]


def _prep_inputs(x, lower_bounds, upper_bounds):
    """fp16 conversion with case-boundary pinning (see module docstring)."""
    F16 = np.float16
    x16 = x.astype(F16)
    l16 = lower_bounds.astype(F16)
    u16 = upper_bounds.astype(F16)
    # l<0 must stay strictly negative AND large enough that tanh(-l/2)
    # cannot round to zero in fp16 (sld's sign carries [l<0] on device).
    l16 = np.where((lower_bounds < 0) & (l16 >= -2.5e-7), F16(-2.5e-7), l16)
    # u>0 must stay strictly positive (case selection uses u<=0).
    u16 = np.where((upper_bounds > 0) & (u16 <= 0), F16(6e-8), u16)
    # u vs Z: the reference jumps at u==Z; keep each element on its f32 side.
    z16 = F16(_Z16)
    below = np.nextafter(z16, F16(0))
    u16 = np.where((upper_bounds >= _Z32) & (u16 < z16), z16, u16)
    u16 = np.where((upper_bounds < _Z32) & (u16 >= z16), below, u16)
    return x16, l16, u16


def _run(x, lower_bounds, upper_bounds, trace=False, **build_kw):
    assert x.shape == (_N,) and x.dtype == np.float32
    nc = _get_nc(**build_kw)
    fd = build_kw.get("fd", 2048)
    nt = _FDT // fd
    x16, l16, u16 = _prep_inputs(x, lower_bounds, upper_bounds)
    shp = (_NCORES, nt, _P, fd)
    packed = np.empty((_NCORES, nt, _P, 3, fd), dtype=np.float16)
    packed[..., 0, :] = l16.reshape(shp)
    packed[..., 1, :] = u16.reshape(shp)
    packed[..., 2, :] = x16.reshape(shp)
    in_maps = [{"pin": packed[c]} for c in range(_NCORES)]
    res = run_bass_kernel_spmd(
        nc, in_maps, core_ids=list(range(_NCORES)), trace=trace
    )
    pout = np.stack([res.results[c]["pout"] for c in range(_NCORES)])
    out = np.ascontiguousarray(pout[..., 0, :]).reshape(-1).astype(np.float32)
    nl = np.ascontiguousarray(pout[..., 1, :]).reshape(-1).astype(np.float32)
    nl = nl * 0.5 - 0.5  # device emits nl in doubled +0.5 space
    nu = np.ascontiguousarray(pout[..., 2, :]).reshape(-1).astype(np.float32)
    nu = nu * 0.5  # device emits nu in doubled space
    return (out, nl, nu), res


def kernel(x, lower_bounds, upper_bounds):
    (out, nl, nu), _ = _run(x, lower_bounds, upper_bounds)
    return (out, nl, nu)


# revision 35
# speedup vs baseline: 1.0256x; 1.0007x over previous
"""DeepPoly SPU transformer — Trainium2 Bass kernel (custom-DVE edition).

Elementwise over N=16777216; sharded across 8 NeuronCores (2M elems each,
viewed as [nt x 128 x fd] fp16).  All wire traffic is fp16 (24MB/core round
trip = ~69.9us at the 360GB/s DMA roofline); the three input streams are
packed into one DRAM tensor (and the three outputs into another) so each
chunk needs one input DMA (+ a split [nl|nu]/[out] pair of output DMAs so
the early streams can fill DMA gaps).  The whole per-element DAG is
collapsed into 3 fused custom-DVE ops + a DVE tensor-tensor max + 2 ACT
transcendentals + an ACT/Pool tail for the `out` stream, so every engine
sits below the DMA roofline (~8.7us per 128x2048 chunk): per full chunk
ACT 7.6us, DVE 7.7us, Pool 6.9us vs DMA 8.7us.  The first tile is split
into quarter/half chunks (and routed through the low-latency all-DVE out
path) to shorten the pipeline fill; the nl max runs at high priority so
its out-DMA is never stuck behind the next chunk's fused ops.  Measured
74894ns vs the ~73.3us model floor (first-DMA latency + DMA busy + final
sem prop); 73363ns with the joint-sweep config (io_bufs=6, out_bufs=5,
tmp_bufs=2, high-priority nl-max and out-DMAs, sigmoid-first ACT order
— the knobs only win jointly).

Math (per element; Z = sqrt(0.5), spu(t) = t^2-0.5 for t>=0 else
sigmoid(-t)-1).  Device emits nl/nu in doubled space (host applies the
constant affine epilogue  nl = 0.5*nlD - 0.5,  nu = 0.5*nuD):

  sld = tanh(-l/2) = 2*(sigmoid(-l) - 0.5)      [ACT; sign(sld) = -sign(l)]
  sgx = sigmoid(x)                              [ACT]
  out = relu(x)^2 - min(sgx, 0.5)               [ACT relu/square + Pool]
  P2  = 2*g2m*(l - g2m/4),  g2m = 2Z*max([l<0], (u+l)/(2Z))   [custom DVE:
        G1 = max([l<0], (u+l)/(2Z));  P2 = G1*(4Z*l - G1)]
  SD  = sld*[u<=0] - BIG*([u>=Z] + [sld<=0])    [custom DVE]
  nuD = max(2*relu(u)^2, sld) + (2*[sld<=0]-1)  [custom DVE]
  nlD = max(P2, SD)                             [DVE tt max]

Case boundaries (l vs 0, u vs 0, u vs Z) are pinned to the f32 side during
host-side fp16 conversion; l<0 is additionally kept <= -2.5e-7 so that
tanh(-l/2) cannot round to +-0 in fp16 (sld's sign carries [l<0] on
device).  Validated vs the f64 reference: relmax_vs_scale ~ 1e-3 on all
three outputs (tolerance 2e-2).
"""

import numpy as np

import concourse.bass as bass
import concourse.bacc as bacc
import concourse.mybir as mybir
from concourse.tile import TileContext
from concourse.bass_utils import run_bass_kernel_spmd
import concourse.dve_ops as dve_ops
from concourse.dve_spec import (
    Spec, Src0, Src1, C0, C1, Zero, One, maxx, minn, relu, sq, lower as _dve_lower,
    _has_src1,
)
from concourse.dve_uop import DveOpSpec

_N = 16777216
_NCORES = 8
_P = 128
_FDT = _N // _NCORES // _P  # 16384 free elems per partition per core

_Z32 = np.float32(np.sqrt(0.5))
_Z16 = float(np.float16(np.sqrt(0.5)))
_INV2Z = float(np.float32(1.0 / (2.0 * float(_Z32))))
_FOURZ = float(np.float32(4.0 * float(_Z32)))
_BIG = 1000.0

_AF = mybir.ActivationFunctionType
_OP = mybir.AluOpType
_F16 = mybir.dt.float16


# --------------------------------------------------------------------------
# custom DVE ops (registered once per process via the documented extension
# point in concourse.dve_ops; names are namespaced to this kernel)
# --------------------------------------------------------------------------

def _register(name, spec):
    if name in dve_ops._SUB_OPCODE_FOR_NAME:
        for op in dve_ops.OPS:
            if op.name == name:
                return op
    row = max(dve_ops._SUB_OPCODE_FOR_NAME.values()) + 1
    assert row < 0x20, "custom-DVE opcode rows exhausted"
    shas = {}
    for ver in ("v3", "v4"):
        u = _dve_lower(spec, ver=ver)
        shas[ver] = DveOpSpec(
            name=name, opcode=row, uops=u, rd1_en=_has_src1(spec)
        ).sha(ver)
    op = dve_ops.DveOp(name, spec, subdim=False, uops_sha=shas)
    dve_ops.OPS.append(op)
    dve_ops._SUB_OPCODE_FOR_NAME[name] = row
    dve_ops.CUSTOM_DVE_SPECS[name] = spec
    return op


def _ref_p2(in0, in1, c0, c1, c2):
    l = in0.astype(np.float32)
    u = in1.astype(np.float32)
    s2s = (l + u) * np.float32(c0)
    g1 = np.maximum((l < 0).astype(np.float32), s2s)
    return g1 * (l * np.float32(c1) - g1)


_g1 = maxx(Src0 < Zero, (Src0 + Src1) * C0)
_SPEC_P2 = Spec(body=_g1 * (Src0 * C1 - _g1), reference=_ref_p2)


def _ref_sd(in0, in1, c0, c1, c2):
    u = in0.astype(np.float32)
    sld = in1.astype(np.float32)
    m1 = (u <= 0).astype(np.float32)
    m2 = (u >= np.float32(c0)).astype(np.float32)
    m3 = (sld <= 0).astype(np.float32)
    return sld * m1 + (m2 + m3) * np.float32(c1)


_SPEC_SD = Spec(
    body=Src1 * (Src0 <= Zero) + ((Src0 >= C0) + (Src1 <= Zero)) * C1,
    reference=_ref_sd,
)


def _ref_nu(in0, in1, c0, c1, c2):
    u = in0.astype(np.float32)
    sld = in1.astype(np.float32)
    r2 = np.maximum(u, 0) ** 2
    c = (sld <= 0).astype(np.float32)
    return np.maximum(r2 + r2, sld) + (c + c - 1.0)


_r2 = sq(relu(Src0))
_c = (Src1 <= Zero)
_SPEC_NU = Spec(body=maxx(_r2 + _r2, Src1) + (_c + _c - One), reference=_ref_nu)


def _ref_out(in0, in1, c0, c1, c2):
    x = in0.astype(np.float32)
    sgx = in1.astype(np.float32)
    return np.maximum(x, 0) ** 2 - np.minimum(sgx, np.float32(c0))


_SPEC_OUT = Spec(body=sq(relu(Src0)) - minn(Src1, C0), reference=_ref_out)

_OP_P2 = _register("SPU_P2_ANT", _SPEC_P2)
_OP_SD = _register("SPU_SD_ANT", _SPEC_SD)
_OP_NU = _register("SPU_NU_ANT", _SPEC_NU)
_OP_OUT = _register("SPU_OUT_ANT", _SPEC_OUT)


# --------------------------------------------------------------------------
# kernel build
# --------------------------------------------------------------------------

def _build_nc(fd=2048, io_bufs=6, out_bufs=5, tmp_bufs=2, ramp="s24e2",
              dve_head=1, dve_tail=0, rx_dve=0, in_q="sp", out_q="sp",
              emit_order="dma_first", skew=3, hp_out=1, split_out=1, split_in=0,
              half_out=0, hp_act=0, hp_max=1, sg_dve=0, act_order=1, dve_every=0,
              rx_set=None, fdt=_FDT):
    """dve_head/dve_tail: how many chunks at each end of the pipeline route
    their `out` stream through the fused DVE op (low latency) instead of the
    ACT/Pool tail (better steady-state balance).  rx_dve: chunks whose
    relu(x) runs as a DVE tensor_scalar instead of ACT."""
    nc = bacc.Bacc(trn_type="TRN2", debug=False, num_devices=_NCORES)
    nt = fdt // fd

    def _q(name):
        return {"sp": nc.sync, "act": nc.scalar, "dve": nc.vector,
                "pool": nc.gpsimd}[name]

    t_in = nc.dram_tensor("pin", [nt, _P, 3, fd], _F16, kind="ExternalInput")
    t_out = nc.dram_tensor("pout", [nt, _P, 3, fd], _F16, kind="ExternalOutput")

    with TileContext(nc) as tc:
        with tc.tile_pool(name="io", bufs=io_bufs) as iop, \
             tc.tile_pool(name="ot", bufs=out_bufs) as otp, \
             tc.tile_pool(name="tmp", bufs=tmp_bufs) as tp:

            if ramp == "end":
                chunks = [(i, 0, fd) for i in range(nt - 1)]
                chunks += [(nt - 1, c, fd // 2) for c in range(0, fd, fd // 2)]
            elif ramp == "end4":
                chunks = [(i, 0, fd) for i in range(nt - 1)]
                chunks += [(nt - 1, c, fd // 4) for c in range(0, fd, fd // 4)]
            elif ramp == "both":
                chunks = [(0, c, fd // 2) for c in range(0, fd, fd // 2)]
                chunks += [(i, 0, fd) for i in range(1, nt - 1)]
                chunks += [(nt - 1, c, fd // 4) for c in range(0, fd, fd // 4)]
            elif ramp == "s4e2":
                chunks = [(0, c, fd // 4) for c in range(0, fd, fd // 4)]
                chunks += [(i, 0, fd) for i in range(1, nt - 1)]
                chunks += [(nt - 1, c, fd // 2) for c in range(0, fd, fd // 2)]
            elif ramp == "s4e4":
                chunks = [(0, c, fd // 4) for c in range(0, fd, fd // 4)]
                chunks += [(i, 0, fd) for i in range(1, nt - 1)]
                chunks += [(nt - 1, c, fd // 4) for c in range(0, fd, fd // 4)]
            elif ramp == "s2e2":
                chunks = [(0, c, fd // 2) for c in range(0, fd, fd // 2)]
                chunks += [(i, 0, fd) for i in range(1, nt - 1)]
                chunks += [(nt - 1, c, fd // 2) for c in range(0, fd, fd // 2)]
            elif ramp == "s24e2":
                chunks = [(0, 0, fd // 4), (0, fd // 4, fd // 4), (0, fd // 2, fd // 2)]
                chunks += [(i, 0, fd) for i in range(1, nt - 1)]
                chunks += [(nt - 1, c, fd // 2) for c in range(0, fd, fd // 2)]
            elif ramp == "s24e24":
                chunks = [(0, 0, fd // 4), (0, fd // 4, fd // 4), (0, fd // 2, fd // 2)]
                chunks += [(i, 0, fd) for i in range(1, nt - 1)]
                chunks += [(nt - 1, 0, fd // 2), (nt - 1, fd // 2, fd // 4),
                           (nt - 1, 3 * fd // 4, fd // 4)]
            elif ramp.startswith("m"):
                # s24 start, split tile <m> (and optionally more) into halves
                msp = {int(c) for c in ramp[1:].split("_")}
                chunks = [(0, 0, fd // 4), (0, fd // 4, fd // 4), (0, fd // 2, fd // 2)]
                for i in range(1, nt - 1):
                    if i in msp:
                        chunks += [(i, 0, fd // 2), (i, fd // 2, fd // 2)]
                    else:
                        chunks.append((i, 0, fd))
                chunks += [(nt - 1, c, fd // 2) for c in range(0, fd, fd // 2)]
            elif ramp == "s42e2":
                chunks = [(0, 0, fd // 2), (0, fd // 2, fd // 4),
                          (0, 3 * fd // 4, fd // 4)]
                chunks += [(i, 0, fd) for i in range(1, nt - 1)]
                chunks += [(nt - 1, c, fd // 2) for c in range(0, fd, fd // 2)]
            elif ramp == "s124e2":
                chunks = [(0, 0, fd // 8), (0, fd // 8, fd // 8),
                          (0, fd // 4, fd // 4), (0, fd // 2, fd // 2)]
                chunks += [(i, 0, fd) for i in range(1, nt - 1)]
                chunks += [(nt - 1, c, fd // 2) for c in range(0, fd, fd // 2)]
            elif ramp == "s24":
                chunks = [(0, 0, fd // 4), (0, fd // 4, fd // 4), (0, fd // 2, fd // 2)]
                chunks += [(i, 0, fd) for i in range(1, nt)]
            elif ramp == "s24e2x":
                chunks = [(0, 0, fd // 4), (0, fd // 4, fd // 4), (0, fd // 2, fd // 2)]
                chunks += [(i, 0, fd) for i in range(1, nt - 2)]
                chunks += [(i, c, fd // 2) for i in (nt - 2, nt - 1)
                           for c in range(0, fd, fd // 2)]
            elif ramp == "s8e2":
                chunks = [(0, c, fd // 8) for c in range(0, fd, fd // 8)]
                chunks += [(i, 0, fd) for i in range(1, nt - 1)]
                chunks += [(nt - 1, c, fd // 2) for c in range(0, fd, fd // 2)]
            else:
                chunks = [(i, 0, fd) for i in range(nt)]

            n = len(chunks)
            st = [None] * n

            def SDMA(ci):
                i, c0, fdc = chunks[ci]
                with tc.high_priority():
                    it = iop.tile([_P, 3 * fdc], _F16, tag="in")
                    itv = it[:].rearrange("p (s f) -> p s f", s=3)
                    if split_in:
                        _q(in_q).dma_start(out=itv[:, 0:2, :],
                                           in_=t_in[i, :, 0:2, c0:c0 + fdc])
                        _q(in_q).dma_start(out=itv[:, 2:3, :],
                                           in_=t_in[i, :, 2:3, c0:c0 + fdc])
                    else:
                        _q(in_q).dma_start(out=itv, in_=t_in[i, :, :, c0:c0 + fdc])
                st[ci] = dict(it=it)

            def S0(ci):
                i, c0, fdc = chunks[ci]
                it = st[ci]["it"]
                l = it[:, 0:fdc]
                x = it[:, 2 * fdc:3 * fdc]
                sld = tp.tile([_P, fdc], _F16, tag="sld")
                sgx = tp.tile([_P, fdc], _F16, tag="sgx")
                if hp_act:
                    with tc.high_priority():
                        nc.scalar.activation(sld[:], l, _AF.Tanh, scale=-0.5)
                        nc.scalar.activation(sgx[:], x, _AF.Sigmoid)
                elif act_order == 1:
                    nc.scalar.activation(sgx[:], x, _AF.Sigmoid)
                    nc.scalar.activation(sld[:], l, _AF.Tanh, scale=-0.5)
                elif act_order == 2:
                    pass  # emitted after rx2 below
                else:
                    nc.scalar.activation(sld[:], l, _AF.Tanh, scale=-0.5)
                    nc.scalar.activation(sgx[:], x, _AF.Sigmoid)
                dve_out = (ci < dve_head or ci >= n - dve_tail or
                           (dve_every and ci % dve_every == 0))
                if act_order == 2 and dve_out:
                    nc.scalar.activation(sld[:], l, _AF.Tanh, scale=-0.5)
                    nc.scalar.activation(sgx[:], x, _AF.Sigmoid)
                if not dve_out:
                    h = fdc // 2 if (half_out and fdc == fd) else 0
                    rx2 = tp.tile([_P, fdc - h], _F16, tag="rx2")
                    if (rx_set is not None and ci in rx_set) or \
                            (rx_dve and (ci % rx_dve == 0)):
                        rx = tp.tile([_P, fdc - h], _F16, tag="rx")
                        nc.vector.tensor_scalar(rx[:], x[:, h:fdc], 0.0, None, _OP.max)
                        nc.scalar.activation(rx2[:], rx[:], _AF.Square)
                    else:
                        nc.scalar.activation(rx2[:], x[:, h:fdc], _AF.Relu)
                        if act_order == 2:
                            nc.scalar.activation(sld[:], l, _AF.Tanh, scale=-0.5)
                            nc.scalar.activation(sgx[:], x, _AF.Sigmoid)
                        nc.scalar.activation(rx2[:], rx2[:], _AF.Square)
                    st[ci].update(rx2=rx2, h=h)
                st[ci].update(sld=sld, sgx=sgx, dve_out=dve_out)

            def S1(ci):
                i, c0, fdc = chunks[ci]
                d = st[ci]
                it = d["it"]
                l3 = it[:, 0:fdc].unsqueeze(1)
                u3 = it[:, fdc:2 * fdc].unsqueeze(1)
                sld3 = d["sld"][:].unsqueeze(1)
                ot = otp.tile([_P, 3 * fdc], _F16, tag="out")
                p2 = tp.tile([_P, fdc], _F16, tag="p2")
                sd = tp.tile([_P, fdc], _F16, tag="sd")
                nc.vector._custom_dve(_OP_P2, out=p2[:].unsqueeze(1), in0=l3,
                                      in1=u3, s0=_INV2Z, s1=_FOURZ)
                nc.vector._custom_dve(_OP_SD, out=sd[:].unsqueeze(1), in0=u3,
                                      in1=sld3, s0=_Z16, s1=-_BIG)
                nc.vector._custom_dve(_OP_NU, out=ot[:, 2 * fdc:3 * fdc].unsqueeze(1),
                                      in0=u3, in1=sld3)
                if d["dve_out"]:
                    x3 = it[:, 2 * fdc:3 * fdc].unsqueeze(1)
                    nc.vector._custom_dve(_OP_OUT, out=ot[:, 0:fdc].unsqueeze(1),
                                          in0=x3, in1=d["sgx"][:].unsqueeze(1),
                                          s0=0.5)
                else:
                    h = d["h"]
                    if h:
                        x3h = it[:, 2 * fdc:2 * fdc + h].unsqueeze(1)
                        nc.vector._custom_dve(
                            _OP_OUT, out=ot[:, 0:h].unsqueeze(1), in0=x3h,
                            in1=d["sgx"][:, 0:h].unsqueeze(1), s0=0.5)
                    sg = tp.tile([_P, fdc - h], _F16, tag="sg")
                    eng = nc.vector if sg_dve else nc.gpsimd
                    eng.tensor_scalar(sg[:], d["sgx"][:, h:fdc], 0.5, None,
                                      _OP.min)
                    st[ci].update(sg=sg)
                st[ci].update(ot=ot, p2=p2, sd=sd)

            def S2(ci):
                i, c0, fdc = chunks[ci]
                d = st[ci]
                ot = d["ot"]
                if not d["dve_out"]:
                    h = d["h"]
                    nc.gpsimd.tensor_tensor(ot[:, h:fdc], d["rx2"][:], d["sg"][:],
                                            _OP.subtract)
                if hp_max:
                    with tc.high_priority():
                        nc.vector.tensor_tensor(ot[:, fdc:2 * fdc], d["p2"][:],
                                                d["sd"][:], _OP.max)
                else:
                    nc.vector.tensor_tensor(ot[:, fdc:2 * fdc], d["p2"][:],
                                            d["sd"][:], _OP.max)
                if split_out and not d["dve_out"]:
                    otv = ot[:].rearrange("p (s f) -> p s f", s=3)
                    _q(out_q).dma_start(
                        out=t_out[i, :, 1:3, c0:c0 + fdc],
                        in_=otv[:, 1:3, :])
                    _q(out_q).dma_start(
                        out=t_out[i, :, 0:1, c0:c0 + fdc],
                        in_=otv[:, 0:1, :])
                elif hp_out:
                    with tc.high_priority():
                        _q(out_q).dma_start(
                            out=t_out[i, :, :, c0:c0 + fdc],
                            in_=ot[:].rearrange("p (s f) -> p s f", s=3))
                else:
                    _q(out_q).dma_start(
                        out=t_out[i, :, :, c0:c0 + fdc],
                        in_=ot[:].rearrange("p (s f) -> p s f", s=3))
                st[ci] = None

            if skew == 0:
                for ci in range(n):
                    SDMA(ci); S0(ci); S1(ci); S2(ci)
            elif skew == 1:
                for k in range(n + 1):
                    if k < n: SDMA(k)
                    if 0 <= k - 1 < n:
                        S0(k - 1); S1(k - 1); S2(k - 1)
            elif skew == 2:
                for k in range(n + 2):
                    if k < n: SDMA(k)
                    if 0 <= k - 1 < n: S0(k - 1)
                    if 0 <= k - 2 < n: S1(k - 2); S2(k - 2)
            else:
                order = {
                    "dma_first": lambda k: [(SDMA, k), (S0, k - 1), (S1, k - 2), (S2, k - 3)],
                    "s2_mid": lambda k: [(SDMA, k), (S0, k - 1), (S2, k - 3), (S1, k - 2)],
                }[emit_order]
                for k in range(n + 3):
                    for fn, ci in order(k):
                        if 0 <= ci < n:
                            fn(ci)

    nc.compile()
    return nc


_NC_CACHE = {}


def _get_nc(**kw):
    key = tuple(sorted(kw.items()))
    if key not in _NC_CACHE:
        _NC_CACHE[key] = _build_nc(**kw)
    return _NC_CACHE[key]


def _prep_inputs(x, lower_bounds, upper_bounds):
    """fp16 conversion with case-boundary pinning (see module docstring)."""
    F16 = np.float16
    x16 = x.astype(F16)
    l16 = lower_bounds.astype(F16)
    u16 = upper_bounds.astype(F16)
    # l<0 must stay strictly negative AND large enough that tanh(-l/2)
    # cannot round to zero in fp16 (sld's sign carries [l<0] on device).
    l16 = np.where((lower_bounds < 0) & (l16 >= -2.5e-7), F16(-2.5e-7), l16)
    # u>0 must stay strictly positive (case selection uses u<=0).
    u16 = np.where((upper_bounds > 0) & (u16 <= 0), F16(6e-8), u16)
    # u vs Z: the reference jumps at u==Z; keep each element on its f32 side.
    z16 = F16(_Z16)
    below = np.nextafter(z16, F16(0))
    u16 = np.where((upper_bounds >= _Z32) & (u16 < z16), z16, u16)
    u16 = np.where((upper_bounds < _Z32) & (u16 >= z16), below, u16)
    return x16, l16, u16


def _run(x, lower_bounds, upper_bounds, trace=False, **build_kw):
    assert x.shape == (_N,) and x.dtype == np.float32
    nc = _get_nc(**build_kw)
    fd = build_kw.get("fd", 2048)
    nt = _FDT // fd
    x16, l16, u16 = _prep_inputs(x, lower_bounds, upper_bounds)
    shp = (_NCORES, nt, _P, fd)
    packed = np.empty((_NCORES, nt, _P, 3, fd), dtype=np.float16)
    packed[..., 0, :] = l16.reshape(shp)
    packed[..., 1, :] = u16.reshape(shp)
    packed[..., 2, :] = x16.reshape(shp)
    in_maps = [{"pin": packed[c]} for c in range(_NCORES)]
    res = run_bass_kernel_spmd(
        nc, in_maps, core_ids=list(range(_NCORES)), trace=trace
    )
    pout = np.stack([res.results[c]["pout"] for c in range(_NCORES)])
    out = np.ascontiguousarray(pout[..., 0, :]).reshape(-1).astype(np.float32)
    nl = np.ascontiguousarray(pout[..., 1, :]).reshape(-1).astype(np.float32)
    nl = nl * 0.5 - 0.5  # device emits nl in doubled +0.5 space
    nu = np.ascontiguousarray(pout[..., 2, :]).reshape(-1).astype(np.float32)
    nu = nu * 0.5  # device emits nu in doubled space
    return (out, nl, nu), res


def kernel(x, lower_bounds, upper_bounds):
    (out, nl, nu), _ = _run(x, lower_bounds, upper_bounds)
    return (out, nl, nu)
